# revision 1
# baseline (speedup 1.0000x reference)
"""Trainium2 Bass kernel for HIVNet GCN message passing (8-core SPMD).

Strategy:
  - Pad N=10000 nodes to 10240 = 80 blocks x 128; core c owns 10 dst-blocks.
  - Per layer: hs = h*rsqrt(deg) (per-node scale), hws = hs @ W[l] computed on
    the owned shard, AllGather of bf16 hws into a DRAM table on every core.
  - Edge aggregation: edges (with self loops) sorted by dst; per dst-block a
    bulk dma_gather pulls the src rows (bf16, 512B each) into SBUF tiles
    [128 edges, 256]; one-hot "sel" matrices (host-built, bf16) reduce each
    128-edge tile onto the 128 dst rows via TensorE matmuls accumulated in
    PSUM.  t = nrm * segsum(hws[src]) applied via per-partition ACT scale.
  - BatchNorm: partial sums/sumsq per core -> 2KB AllReduce -> scale/shift
    broadcast via rank-1 TensorE matmul; relu + residual on DVE.
  - Readout: graph mean-pool via one-hot pool matrices (transposed layout so
    MLP runs with weights as lhsT), 257-row AllReduce, 3-layer MLP on core 0.
"""

import sys

sys.path.insert(0, "/opt/trn_rl_repo")

from contextlib import ExitStack

import numpy as np
import ml_dtypes

from concourse import bass, mybir, bacc, tile, library_config
from concourse.bass_utils import run_bass_kernel_spmd
from concourse.masks import make_identity

NCORE = 8
P = 128
H = 256
L = 4
NF = 9
G = 256
N = 10000
BPC = 10                # dst blocks per core
NPC = BPC * P           # 1280 nodes per core
NPAD = NCORE * NPC      # 10240
BN_EPS = 1e-5

f32 = mybir.dt.float32
bf16 = mybir.dt.bfloat16
i16 = mybir.dt.int16
bfnp = ml_dtypes.bfloat16

FT = mybir.ActivationFunctionType
OP = mybir.AluOpType

_compiled = {}


# --------------------------------------------------------------------------
# host-side structural preprocessing (sorting / padding / one-hot layout)
# --------------------------------------------------------------------------

def _preprocess(x, edge_index, batch_ids, emb, W, gamma, beta,
                mlp_W1, mlp_b1, mlp_W2, mlp_b2, mlp_W3, mlp_b3):
    src = np.asarray(edge_index[0], np.int64)
    dst = np.asarray(edge_index[1], np.int64)
    # self loops for every real node (weight nrm[d]^2 == nrm[d]*nrm[d] folds in)
    src_all = np.concatenate([src, np.arange(N, dtype=np.int64)])
    dst_all = np.concatenate([dst, np.arange(N, dtype=np.int64)])
    order = np.argsort(dst_all, kind="stable")
    s_sorted = src_all[order].astype(np.int64)
    d_sorted = dst_all[order]

    deg = np.bincount(dst_all, minlength=NPAD).astype(np.float64)  # incl self
    nblk = NCORE * BPC
    cnt_blk = np.bincount(d_sorted // P, minlength=nblk)
    T_blk = int(np.ceil(cnt_blk.max() / P))
    NI = T_blk * P
    NIB = NI // 16

    idx_slots = np.zeros((nblk, NI), np.int16)
    dloc = np.full((nblk, NI), -1, np.int32)
    starts = np.searchsorted(d_sorted, np.arange(nblk) * P)
    ends = np.searchsorted(d_sorted, (np.arange(nblk) + 1) * P)
    for g in range(nblk):
        c = ends[g] - starts[g]
        idx_slots[g, :c] = s_sorted[starts[g]:ends[g]]
        dloc[g, :c] = d_sorted[starts[g]:ends[g]] - g * P

    # one-hot sel: [blk, T_blk, 128 slots, 128 dst_local] bf16
    sel = (dloc.reshape(nblk, T_blk, P)[..., None]
           == np.arange(P, dtype=np.int32)).astype(bfnp)

    # graph pool one-hot [node, graph]
    bids = np.asarray(batch_ids, np.int64)
    psel_full = np.zeros((NPAD, G), np.float32)
    psel_full[np.arange(N), bids] = 1.0

    x_np = np.zeros((NPAD, NF), np.float32)
    x_np[:N] = np.asarray(x, np.float64)

    # shared parameter tensors (layout for device)
    Wf = np.asarray(W, np.float32)                       # [L,H,H]
    W_lhsT = Wf.reshape(L, 2, P, H).transpose(2, 0, 1, 3).reshape(P, L * 2 * H)
    gb = np.concatenate([np.asarray(gamma, np.float32).reshape(-1),
                         np.asarray(beta, np.float32).reshape(-1)])[None, :]
    embf = np.asarray(emb, np.float32)
    emb0 = np.ascontiguousarray(embf[:, 0, :])
    emb1 = np.ascontiguousarray(embf[:, 1, :])
    w1 = np.asarray(mlp_W1, np.float32).reshape(2, P, P).transpose(1, 0, 2).reshape(P, 2 * P)
    w2 = np.asarray(mlp_W2, np.float32)                  # [128,64]
    w3 = np.asarray(mlp_W3, np.float32)                  # [64,1]
    b1 = np.asarray(mlp_b1, np.float32).reshape(P, 1)
    b2 = np.asarray(mlp_b2, np.float32).reshape(64, 1)
    b3 = np.asarray(mlp_b3, np.float32).reshape(1, 1)

    in_maps = []
    for c in range(NCORE):
        lo, hi = c * NPC, (c + 1) * NPC
        gsl = slice(c * BPC, (c + 1) * BPC)

        selc = sel[gsl].reshape(BPC * T_blk, P, P)
        selc = np.ascontiguousarray(selc.transpose(1, 0, 2)).reshape(P, BPC * T_blk * P)

        idxc = idx_slots[gsl].reshape(BPC, NI // 16, 16)
        idxc = idxc.transpose(0, 2, 1)                    # [BPC, 16, NI/16]
        idxc = np.tile(idxc, (1, 8, 1))                   # replicate to 128 parts
        idxc = np.ascontiguousarray(idxc.transpose(1, 0, 2)).reshape(P, BPC * NIB)

        degc = deg[lo:hi].reshape(BPC, P).T               # [P, BPC]
        maskc = (degc > 0).astype(np.float32)
        degc = np.maximum(degc, 1.0).astype(np.float32)

        pselc = psel_full[lo:hi].reshape(BPC, P, G)
        pselc = np.ascontiguousarray(pselc.transpose(1, 0, 2)).reshape(P, BPC * G)

        xTc = np.ascontiguousarray(x_np[lo:hi].T)         # [NF, NPC]

        in_maps.append(dict(
            selw=selc.astype(bfnp), idx=idxc.astype(np.int16),
            xT=xTc, deg=degc, mask=maskc, psel=pselc,
            W=W_lhsT.astype(bfnp), gb=gb, emb0=emb0, emb1=emb1,
            w1=w1, w2=w2, w3=w3, b1=b1, b2=b2, b3=b3,
        ))
    return T_blk, in_maps


# --------------------------------------------------------------------------
# device program
# --------------------------------------------------------------------------

def _build(T_blk, variant='full'):
    NI = T_blk * P
    NIB = NI // 16
    nc = bacc.Bacc(None, target_bir_lowering=False)

    d_sel = nc.dram_tensor("selw", [P, BPC * T_blk * P], bf16, kind="ExternalInput")
    d_idx = nc.dram_tensor("idx", [P, BPC * NIB], i16, kind="ExternalInput")
    d_xT = nc.dram_tensor("xT", [NF, NPC], f32, kind="ExternalInput")
    d_deg = nc.dram_tensor("deg", [P, BPC], f32, kind="ExternalInput")
    d_mask = nc.dram_tensor("mask", [P, BPC], f32, kind="ExternalInput")
    d_psel = nc.dram_tensor("psel", [P, BPC * G], f32, kind="ExternalInput")
    d_W = nc.dram_tensor("W", [P, L * 2 * H], bf16, kind="ExternalInput")
    d_gb = nc.dram_tensor("gb", [1, 2 * L * H], f32, kind="ExternalInput")
    d_emb0 = nc.dram_tensor("emb0", [NF, H], f32, kind="ExternalInput")
    d_emb1 = nc.dram_tensor("emb1", [NF, H], f32, kind="ExternalInput")
    d_w1 = nc.dram_tensor("w1", [P, 2 * P], f32, kind="ExternalInput")
    d_w2 = nc.dram_tensor("w2", [P, 64], f32, kind="ExternalInput")
    d_w3 = nc.dram_tensor("w3", [64, 1], f32, kind="ExternalInput")
    d_b1 = nc.dram_tensor("b1", [P, 1], f32, kind="ExternalInput")
    d_b2 = nc.dram_tensor("b2", [64, 1], f32, kind="ExternalInput")
    d_b3 = nc.dram_tensor("b3", [1, 1], f32, kind="ExternalInput")
    d_out = nc.dram_tensor("out", [1, G], f32, kind="ExternalOutput")

    rg = [list(range(NCORE))]

    with tile.TileContext(nc) as tc, ExitStack() as ctx:
        pers = ctx.enter_context(tc.tile_pool(name="pers", bufs=1))
        psA = ctx.enter_context(tc.tile_pool(name="psA", bufs=2, space="PSUM"))
        psB = ctx.enter_context(tc.tile_pool(name="psB", bufs=2, space="PSUM"))
        psC = ctx.enter_context(tc.tile_pool(name="psC", bufs=1, space="PSUM"))
        gpool = ctx.enter_context(tc.tile_pool(name="gpool", bufs=2))
        work = ctx.enter_context(tc.tile_pool(name="work", bufs=2))
        stream = ctx.enter_context(tc.tile_pool(name="stream", bufs=2))
        dram = ctx.enter_context(tc.tile_pool(name="dram", bufs=2, space="DRAM"))

        # ---- persistent SBUF state -------------------------------------
        sel_sb = pers.tile([P, BPC * T_blk * P], bf16, tag="sel")
        idx_sb = pers.tile([P, BPC * NIB], i16, tag="idx")
        deg_sb = pers.tile([P, BPC], f32, tag="deg")
        mask_sb = pers.tile([P, BPC], f32, tag="mask")
        W_sb = pers.tile([P, L * 2 * H], bf16, tag="W")
        gb_sb = pers.tile([1, 2 * L * H], f32, tag="gb")
        emb0_sb = pers.tile([NF, H], f32, tag="emb0")
        emb1_sb = pers.tile([NF, H], f32, tag="emb1")
        w1_sb = pers.tile([P, 2 * P], f32, tag="w1")
        w2_sb = pers.tile([P, 64], f32, tag="w2")
        w3_sb = pers.tile([64, 1], f32, tag="w3")
        b1_sb = pers.tile([P, 1], f32, tag="b1")
        b2_sb = pers.tile([64, 1], f32, tag="b2")
        b3_sb = pers.tile([1, 1], f32, tag="b3")

        h_sb = pers.tile([P, BPC * H], f32, tag="h")
        hsT_sb = pers.tile([P, BPC * 2 * P], bf16, tag="hsT")
        hws_sb = pers.tile([P, BPC * H], bf16, tag="hws")
        t_all = pers.tile([P, BPC * H], f32, tag="t_all")
        nrm_sb = pers.tile([P, BPC], f32, tag="nrm")
        acc_s = pers.tile([P, H], f32, tag="acc_s")
        acc_q = pers.tile([P, H], f32, tag="acc_q")
        D_sb = pers.tile([NF, H], f32, tag="D")
        base_rep = pers.tile([P, H], f32, tag="base_rep")
        a_rep = pers.tile([P, H], f32, tag="a_rep")
        c_rep = pers.tile([P, H], f32, tag="c_rep")
        ident_bf = pers.tile([P, P], bf16, tag="ident")
        ones9 = pers.tile([NF, 1], f32, tag="ones9")
        ones1 = pers.tile([1, P], f32, tag="ones1")
        ones128 = pers.tile([P, 1], f32, tag="ones128")
        stv = pers.tile([1, 2 * H], f32, tag="stv")
        scal = pers.tile([1, 8 * H], f32, tag="scal")

        # ---- DRAM bounce buffers ---------------------------------------
        ag_in = dram.tile([NPC, H], bf16, tag="ag_in")
        ag_out = dram.tile([NPAD, H], bf16, tag="ag_out")
        ar_in = dram.tile([1, 2 * H], f32, tag="ar_in")
        ar_out = dram.tile([1, 2 * H], f32, tag="ar_out")
        pr_in = dram.tile([2 * P + 1, G], f32, tag="pr_in")
        pr_out = dram.tile([2 * P + 1, G], f32, tag="pr_out")

        # ---- input loads ------------------------------------------------
        for t, d in [(sel_sb, d_sel), (idx_sb, d_idx),
                     (deg_sb, d_deg), (mask_sb, d_mask),
                     (W_sb, d_W), (gb_sb, d_gb), (emb0_sb, d_emb0),
                     (emb1_sb, d_emb1), (w1_sb, d_w1), (w2_sb, d_w2),
                     (w3_sb, d_w3), (b1_sb, d_b1), (b2_sb, d_b2),
                     (b3_sb, d_b3)]:
            nc.sync.dma_start(out=t[:], in_=d[:])

        nc.gpsimd.load_library(library_config.mlp)
        make_identity(nc, ident_bf[:])
        nc.vector.memset(ones9[:], 1.0)
        nc.vector.memset(ones1[:], 1.0)
        nc.vector.memset(ones128[:], 1.0)

        # nrm = rsqrt(deg) * mask
        rdeg = work.tile([P, BPC], f32, tag="rdeg")
        nc.vector.reciprocal(out=rdeg[:], in_=deg_sb[:])
        nc.scalar.activation(out=rdeg[:], in_=rdeg[:], func=FT.Sqrt)
        nc.vector.tensor_tensor(out=nrm_sb[:], in0=rdeg[:], in1=mask_sb[:], op=OP.mult)

        # encoder prep: D = emb1 - emb0 ; base = ones9^T @ emb0, broadcast
        nc.vector.tensor_tensor(out=D_sb[:], in0=emb1_sb[:], in1=emb0_sb[:], op=OP.subtract)
        ps_b = psB.tile([1, H], f32, tag="vec")
        nc.tensor.matmul(out=ps_b[:], lhsT=ones9[:], rhs=emb0_sb[:], start=True, stop=True)
        bvec = scal[:, 0:H]
        nc.vector.tensor_copy(out=bvec, in_=ps_b[:])
        ps_br = psB.tile([P, H], f32, tag="vec")
        nc.tensor.matmul(out=ps_br[:], lhsT=ones1[:], rhs=bvec, start=True, stop=True)
        nc.vector.tensor_copy(out=base_rep[:], in_=ps_br[:])

        def hslice(nb):
            return h_sb[:, nb * H:(nb + 1) * H]

        def emit_hs_transpose(nb):
            """hs = h*nrm (bf16), transpose both 128-halves into hsT_sb."""
            hs_bf = work.tile([P, H], bf16, tag="hs_bf")
            nc.vector.tensor_scalar_mul(hs_bf[:], hslice(nb), nrm_sb[:, nb:nb + 1])
            for k in range(2):
                pst = psB.tile([P, P], bf16, tag="pst")
                nc.tensor.transpose(out=pst[:], in_=hs_bf[:, k * P:(k + 1) * P],
                                    identity=ident_bf[:])
                nc.vector.tensor_copy(out=hsT_sb[:, (nb * 2 + k) * P:(nb * 2 + k + 1) * P],
                                      in_=pst[:])

        # encoder: h0 = base + xT^T @ D  (per block)
        for nb in range(BPC):
            xT_t = stream.tile([NF, P], f32, tag="xT_t")
            nc.sync.dma_start(out=xT_t[:], in_=d_xT[:, nb * P:(nb + 1) * P])
            ps_h = psA.tile([P, H], f32, tag="mm")
            nc.tensor.matmul(out=ps_h[:], lhsT=xT_t[:],
                             rhs=D_sb[:], start=True, stop=True)
            nc.vector.tensor_tensor(out=hslice(nb), in0=ps_h[:], in1=base_rep[:], op=OP.add)
            emit_hs_transpose(nb)

        if variant == "enc":
            nc.sync.dma_start(out=d_out[:], in_=h_sb[0:1, 0:G])
        # ---- layers -----------------------------------------------------
        nlayers = 0 if variant == "enc" else (1 if variant in ("ag", "gat", "agg", "l1") else L)
        for l in range(nlayers):
            # GEMM hws = hs @ W[l]  (lhsT = hsT halves, rhs = W k-halves)
            for nb in range(BPC):
                ps_g = psA.tile([P, H], f32, tag="mm")
                for k in range(2):
                    nc.tensor.matmul(
                        out=ps_g[:],
                        lhsT=hsT_sb[:, (nb * 2 + k) * P:(nb * 2 + k + 1) * P],
                        rhs=W_sb[:, (l * 2 + k) * H:(l * 2 + k + 1) * H],
                        start=(k == 0), stop=(k == 1))
                nc.vector.tensor_copy(out=hws_sb[:, nb * H:(nb + 1) * H], in_=ps_g[:])
                nc.sync.dma_start(out=ag_in[nb * P:(nb + 1) * P, :],
                                  in_=hws_sb[:, nb * H:(nb + 1) * H])
            nc.gpsimd.collective_compute(
                "AllGather", OP.bypass, replica_groups=rg,
                ins=[ag_in[:]], outs=[ag_out[:]])
            if variant == "ag":
                sbtmp = work.tile([1, G], bf16, tag="dbg")
                nc.sync.dma_start(out=sbtmp[:], in_=ag_out[0:1, 0:G])
                sbtmp2 = work.tile([1, G], f32, tag="dbg2")
                nc.vector.tensor_copy(out=sbtmp2[:], in_=sbtmp[:])
                nc.sync.dma_start(out=d_out[:], in_=sbtmp2[:])
                break

            nc.vector.memset(acc_s[:], 0.0)
            nc.vector.memset(acc_q[:], 0.0)

            T0 = (T_blk + 1) // 2
            chunks = [(0, T0), (T0, T_blk)]
            for nb in range(BPC):
                gts = []
                for (j0, j1) in chunks:
                    gath = gpool.tile([P, T0 * H], bf16, tag="gath")
                    nc.gpsimd.dma_gather(
                        out_ap=gath[:, :(j1 - j0) * H].rearrange("p (t h) -> p t h", h=H),
                        in_ap=ag_out[:],
                        idxs_ap=idx_sb[:, nb * NIB + j0 * 8:nb * NIB + j1 * 8],
                        num_idxs=(j1 - j0) * P, num_idxs_reg=(j1 - j0) * P,
                        elem_size=H, single_packet=False)
                    gts.append(gath)
                if variant == "gat":
                    gtmp = work.tile([1, G], bf16, tag="dbg")
                    nc.vector.tensor_copy(out=gtmp[:], in_=gts[0][0:1, 0:G])
                    gtmp2 = work.tile([1, G], f32, tag="dbg2")
                    nc.vector.tensor_copy(out=gtmp2[:], in_=gtmp[:])
                    nc.sync.dma_start(out=d_out[:], in_=gtmp2[:])
                    break
                ps_t = psA.tile([P, H], f32, tag="mm")
                for j in range(T_blk):
                    ti = nb * T_blk + j
                    ci = 0 if j < T0 else 1
                    jj = j if j < T0 else j - T0
                    nc.tensor.matmul(
                        out=ps_t[:],
                        lhsT=sel_sb[:, ti * P:(ti + 1) * P],
                        rhs=gts[ci][:, jj * H:(jj + 1) * H],
                        start=(j == 0), stop=(j == T_blk - 1))
                tsl = t_all[:, nb * H:(nb + 1) * H]
                nc.scalar.activation(out=tsl, in_=ps_t[:], func=FT.Copy,
                                     scale=nrm_sb[:, nb:nb + 1])
                sq = work.tile([P, H], f32, tag="tmp")
                nc.vector.tensor_tensor(out=sq[:], in0=tsl, in1=tsl, op=OP.mult)
                nc.vector.tensor_tensor(out=acc_s[:], in0=acc_s[:], in1=tsl, op=OP.add)
                nc.vector.tensor_tensor(out=acc_q[:], in0=acc_q[:], in1=sq[:], op=OP.add)

            if variant == "gat":
                break
            if variant == "agg":
                nc.sync.dma_start(out=d_out[:], in_=t_all[0:1, 0:G])
                break
            # stats: cross-partition reduce + AllReduce
            ps_s = psB.tile([1, 2 * H], f32, tag="vec")
            nc.tensor.matmul(out=ps_s[:, 0:H], lhsT=ones128[:], rhs=acc_s[:],
                             start=True, stop=True)
            nc.tensor.matmul(out=ps_s[:, H:2 * H], lhsT=ones128[:], rhs=acc_q[:],
                             start=True, stop=True)
            st_sb = scal[:, 6 * H:8 * H]
            nc.vector.tensor_copy(out=st_sb, in_=ps_s[:])
            nc.sync.dma_start(out=ar_in[:], in_=st_sb)
            nc.gpsimd.collective_compute(
                "AllReduce", OP.add, replica_groups=rg,
                ins=[ar_in[:]], outs=[ar_out[:]])
            nc.sync.dma_start(out=stv[:], in_=ar_out[:])

            # a = gamma*istd ; c = beta - mu*a   (all [1,H] lanes)
            mu = scal[:, H:2 * H]
            var = scal[:, 2 * H:3 * H]
            av = scal[:, 3 * H:4 * H]
            cv = scal[:, 4 * H:5 * H]
            msq = scal[:, 5 * H:6 * H]
            nc.vector.tensor_scalar_mul(mu, stv[:, 0:H], 1.0 / N)
            nc.vector.tensor_scalar_mul(var, stv[:, H:2 * H], 1.0 / N)
            nc.vector.tensor_tensor(out=msq, in0=mu, in1=mu, op=OP.mult)
            nc.vector.tensor_tensor(out=var, in0=var, in1=msq, op=OP.subtract)
            nc.vector.tensor_scalar_add(var, var, BN_EPS)
            nc.vector.reciprocal(out=var, in_=var)
            nc.scalar.activation(out=var, in_=var, func=FT.Sqrt)  # istd
            nc.vector.tensor_tensor(out=av, in0=var,
                                    in1=gb_sb[:, l * H:(l + 1) * H], op=OP.mult)
            nc.vector.tensor_tensor(out=msq, in0=mu, in1=av, op=OP.mult)
            nc.vector.tensor_tensor(out=cv, in0=gb_sb[:, (L + l) * H:(L + l + 1) * H],
                                    in1=msq, op=OP.subtract)
            ps_a = psB.tile([P, H], f32, tag="vec")
            nc.tensor.matmul(out=ps_a[:], lhsT=ones1[:], rhs=av, start=True, stop=True)
            nc.vector.tensor_copy(out=a_rep[:], in_=ps_a[:])
            ps_c = psB.tile([P, H], f32, tag="vec")
            nc.tensor.matmul(out=ps_c[:], lhsT=ones1[:], rhs=cv, start=True, stop=True)
            nc.vector.tensor_copy(out=c_rep[:], in_=ps_c[:])

            # h = relu(t*a + c) + h ; prepare hsT for next layer
            for nb in range(BPC):
                tsl = t_all[:, nb * H:(nb + 1) * H]
                u = work.tile([P, H], f32, tag="tmp")
                nc.vector.tensor_tensor(out=u[:], in0=tsl, in1=a_rep[:], op=OP.mult)
                nc.vector.tensor_tensor(out=u[:], in0=u[:], in1=c_rep[:], op=OP.add)
                r = work.tile([P, H], f32, tag="tmp2")
                nc.scalar.activation(out=r[:], in_=u[:], func=FT.Relu)
                nc.vector.tensor_tensor(out=hslice(nb), in0=hslice(nb), in1=r[:], op=OP.add)
                if l < L - 1:
                    emit_hs_transpose(nb)

        if variant == "l1":
            nc.sync.dma_start(out=d_out[:], in_=h_sb[0:1, 0:G])
        skip_pool = variant in ("enc", "ag", "gat", "agg", "l1")
        # ---- pooling ----------------------------------------------------
        if not skip_pool:
            ps_p0 = psC.tile([P, G], f32, tag="p0")
            ps_p1 = psC.tile([P, G], f32, tag="p1")
            ps_pc = psB.tile([1, G], f32, tag="vec")
            for nb in range(BPC):
                psel_t = stream.tile([P, G], f32, tag="psel_t")
                nc.sync.dma_start(out=psel_t[:], in_=d_psel[:, nb * G:(nb + 1) * G])
                pssl = psel_t[:]
                nc.tensor.matmul(out=ps_p0[:], lhsT=h_sb[:, nb * H:nb * H + P],
                                 rhs=pssl, start=(nb == 0), stop=(nb == BPC - 1))
                nc.tensor.matmul(out=ps_p1[:], lhsT=h_sb[:, nb * H + P:(nb + 1) * H],
                                 rhs=pssl, start=(nb == 0), stop=(nb == BPC - 1))
                nc.tensor.matmul(out=ps_pc[:], lhsT=ones128[:],
                                 rhs=pssl, start=(nb == 0), stop=(nb == BPC - 1))
            g0 = work.tile([P, G], f32, tag="g0")
            g1 = work.tile([P, G], f32, tag="g1")
            cnt = scal[:, 0:G]
            nc.vector.tensor_copy(out=g0[:], in_=ps_p0[:])
            nc.vector.tensor_copy(out=g1[:], in_=ps_p1[:])
            nc.vector.tensor_copy(out=cnt, in_=ps_pc[:])
            nc.sync.dma_start(out=pr_in[0:P, :], in_=g0[:])
            nc.sync.dma_start(out=pr_in[P:2 * P, :], in_=g1[:])
            nc.sync.dma_start(out=pr_in[2 * P:2 * P + 1, :], in_=cnt)
            nc.gpsimd.collective_compute(
                "AllReduce", OP.add, replica_groups=rg,
                ins=[pr_in[:]], outs=[pr_out[:]])
            nc.sync.dma_start(out=g0[:], in_=pr_out[0:P, :])
            nc.sync.dma_start(out=g1[:], in_=pr_out[P:2 * P, :])
            nc.sync.dma_start(out=cnt, in_=pr_out[2 * P:2 * P + 1, :])
            nc.vector.tensor_scalar_max(cnt, cnt, 1.0)
            nc.vector.reciprocal(out=cnt, in_=cnt)
            ps_r = psB.tile([P, G], f32, tag="vec")
            nc.tensor.matmul(out=ps_r[:], lhsT=ones1[:], rhs=cnt, start=True, stop=True)
            rc_rep = work.tile([P, G], f32, tag="rc_rep")
            nc.vector.tensor_copy(out=rc_rep[:], in_=ps_r[:])
            nc.vector.tensor_tensor(out=g0[:], in0=g0[:], in1=rc_rep[:], op=OP.mult)
            nc.vector.tensor_tensor(out=g1[:], in0=g1[:], in1=rc_rep[:], op=OP.mult)

            # MLP head (transposed: weights are lhsT, graphs along free dim)
            ps1 = psB.tile([P, G], f32, tag="vec")
            nc.tensor.matmul(out=ps1[:], lhsT=w1_sb[:, 0:P], rhs=g0[:], start=True, stop=False)
            nc.tensor.matmul(out=ps1[:], lhsT=w1_sb[:, P:2 * P], rhs=g1[:], start=False, stop=True)
            y1 = work.tile([P, G], f32, tag="y1")
            nc.scalar.activation(out=y1[:], in_=ps1[:], func=FT.Relu, bias=b1_sb[:, 0:1])
            ps2 = psB.tile([64, G], f32, tag="vec")
            nc.tensor.matmul(out=ps2[:], lhsT=w2_sb[:], rhs=y1[:], start=True, stop=True)
            y2 = work.tile([64, G], f32, tag="y2")
            nc.scalar.activation(out=y2[:], in_=ps2[:], func=FT.Relu, bias=b2_sb[:, 0:1])
            ps3 = psB.tile([1, G], f32, tag="vec")
            nc.tensor.matmul(out=ps3[:], lhsT=w3_sb[:], rhs=y2[:], start=True, stop=True)
            y3 = work.tile([1, G], f32, tag="y3")
            nc.vector.tensor_scalar_add(y3[:], ps3[:], b3_sb[0:1, 0:1])
            nc.sync.dma_start(out=d_out[:], in_=y3[:])

    nc.compile()
    return nc


# --------------------------------------------------------------------------
# entry point
# --------------------------------------------------------------------------

def kernel(x, edge_index, batch_ids, emb, W, b, gamma, beta,
           mlp_W1, mlp_b1, mlp_W2, mlp_b2, mlp_W3, mlp_b3,
           _trace=False, _trace_kwargs=None):
    # NB: reference BN subtracts the per-channel mean, so the additive bias b
    # cancels exactly and is not needed by the device program.
    T_blk, in_maps = _preprocess(x, edge_index, batch_ids, emb, W, gamma, beta,
                                 mlp_W1, mlp_b1, mlp_W2, mlp_b2, mlp_W3, mlp_b3)
    import os
    variant = os.environ.get("KVARIANT", "full")
    key = (T_blk, variant)
    if key not in _compiled:
        _compiled[key] = _build(T_blk, variant)
    nc = _compiled[key]
    kw = {}
    if _trace:
        kw = dict(trace=True, **(_trace_kwargs or {}))
    res = run_bass_kernel_spmd(nc, in_maps, core_ids=list(range(NCORE)), **kw)
    out = np.asarray(res.results[0]["out"], np.float32).reshape(G, 1)
    kernel._last_results = res
    return out



# revision 8
# speedup vs baseline: 3.3683x; 3.3683x over previous
"""Trainium2 Bass kernel for HIVNet GCN message passing (8-core SPMD).

Strategy (v2 — dense block-pair aggregation, no per-edge DMA):
  - Pad N=10000 nodes to 10240 = 80 blocks x 128; core c owns 10 dst-blocks.
  - Per layer: hws = (h*rsqrt(deg)) @ W[l] on the owned shard, cast to
    fp8e4m3 (x32) and AllGather'd into a DRAM table; each core loads the
    full [10240, 256] fp8 table into SBUF (20 KB/partition).
  - Edge aggregation: host builds a dense one-hot/count matrix per
    (src-block-pair, dst-block): sel[p, i, d] = #edges from node
    (2k+i)*128+p to dst d (self loops included).  TensorE DoubleRow fp8
    matmuls contract 256 src rows per pass: ps_t[nb] += sel_k^T @ tab_k
    accumulated over the 40 pairs.  Two waves of 5 dst-blocks share the
    8 PSUM banks with the stats accumulator.
  - BatchNorm: t|t^2 packed [128,512] per block, one TensorE ones^T
    matmul accumulates sums; 2KB AllGather + 8-row sum replaces the
    AllReduce; scale/shift broadcast via rank-1 matmuls; apply on DVE.
  - Readout: graph mean-pool via one-hot matrices with 1/cnt folded in
    (host-side), 2*128-row AllReduce, 3-layer MLP, output on every core.
"""

import sys

sys.path.insert(0, "/opt/trn_rl_repo")

from contextlib import ExitStack

import numpy as np
import ml_dtypes

from concourse import bass, mybir, bacc, tile
from concourse.bass_utils import run_bass_kernel_spmd
from concourse.masks import make_identity

NCORE = 8
P = 128
H = 256
L = 4
NF = 9
G = 256
N = 10000
BPC = 10                # dst blocks per core
NPC = BPC * P           # 1280 nodes per core
NPAD = NCORE * NPC      # 10240
NBLK = NPAD // P        # 80 src blocks
NPAIR = NBLK // 2       # 40 src block pairs (DoubleRow K=256)
BN_EPS = 1e-5
FP8S = 32.0             # fp8 table scale

f32 = mybir.dt.float32
bf16 = mybir.dt.bfloat16
fp8 = mybir.dt.float8e4
fp8np = mybir.dt.np(mybir.dt.float8e4)
bfnp = ml_dtypes.bfloat16

FT = mybir.ActivationFunctionType
OP = mybir.AluOpType
DR = mybir.MatmulPerfMode.DoubleRow

_compiled = {}


# --------------------------------------------------------------------------
# host-side structural preprocessing
# --------------------------------------------------------------------------

def _preprocess(x, edge_index, batch_ids, emb, W, gamma, beta,
                mlp_W1, mlp_b1, mlp_W2, mlp_b2, mlp_W3, mlp_b3):
    src = np.asarray(edge_index[0], np.int64)
    dst = np.asarray(edge_index[1], np.int64)
    # self loops for every real node (weight nrm[d]^2 folds in via table)
    src_all = np.concatenate([src, np.arange(N, dtype=np.int64)])
    dst_all = np.concatenate([dst, np.arange(N, dtype=np.int64)])

    deg = np.bincount(dst_all, minlength=NPAD).astype(np.float64)  # incl self

    # dense per-core sel: [128 p, NPAIR, BPC, 2, 128 d] edge counts
    p_idx = (src_all % P).astype(np.int64)
    blk = src_all // P
    k_idx = blk // 2
    i_idx = blk % 2
    core = dst_all // NPC
    nb_idx = (dst_all % NPC) // P
    d_idx = dst_all % P

    sels = []
    for c in range(NCORE):
        m = core == c
        selc = np.zeros((P, NPAIR, BPC, 2, P), np.float32)
        np.add.at(selc, (p_idx[m], k_idx[m], nb_idx[m], i_idx[m], d_idx[m]), 1.0)
        sels.append(selc.reshape(P, NPAIR * BPC * 2 * P).astype(fp8np))

    # graph pool one-hot with 1/count folded in (counts are structural)
    bids = np.asarray(batch_ids, np.int64)
    cnt = np.bincount(bids, minlength=G).astype(np.float32)
    inv = 1.0 / np.maximum(cnt, 1.0)
    psel_full = np.zeros((NPAD, G), np.float32)
    psel_full[np.arange(N), bids] = inv[bids]

    x_np = np.zeros((NPAD, NF), np.float32)
    x_np[:N] = np.asarray(x, np.float64)

    # shared parameter tensors (layout for device)
    Wf = np.asarray(W, np.float32)                       # [L,H,H]
    W_lhsT = Wf.reshape(L, 2, P, H).transpose(2, 0, 1, 3).reshape(P, L * 2 * H)
    gb = np.concatenate([np.asarray(gamma, np.float32).reshape(-1),
                         np.asarray(beta, np.float32).reshape(-1)])[None, :]
    embf = np.asarray(emb, np.float32)
    emb0 = np.ascontiguousarray(embf[:, 0, :])
    emb1 = np.ascontiguousarray(embf[:, 1, :])
    w1 = np.asarray(mlp_W1, np.float32).reshape(2, P, P).transpose(1, 0, 2).reshape(P, 2 * P)
    w2 = np.asarray(mlp_W2, np.float32)                  # [128,64]
    w3 = np.asarray(mlp_W3, np.float32)                  # [64,1]
    b1 = np.asarray(mlp_b1, np.float32).reshape(P, 1)
    b2 = np.asarray(mlp_b2, np.float32).reshape(64, 1)
    b3 = np.asarray(mlp_b3, np.float32).reshape(1, 1)

    in_maps = []
    for c in range(NCORE):
        lo, hi = c * NPC, (c + 1) * NPC

        degc = deg[lo:hi].reshape(BPC, P).T               # [P, BPC]
        maskc = (degc > 0).astype(np.float32)
        degc = np.maximum(degc, 1.0).astype(np.float32)

        pselc = psel_full[lo:hi].reshape(BPC, P, G)
        pselc = np.ascontiguousarray(pselc.transpose(1, 0, 2)).reshape(P, BPC * G)

        xTc = np.ascontiguousarray(x_np[lo:hi].T)         # [NF, NPC]

        in_maps.append(dict(
            selw=sels[c], xT=xTc, deg=degc, mask=maskc, psel=pselc,
            W=W_lhsT.astype(bfnp), gb=gb, emb0=emb0, emb1=emb1,
            w1=w1, w2=w2, w3=w3, b1=b1, b2=b2, b3=b3,
        ))
    return in_maps


# --------------------------------------------------------------------------
# device program
# --------------------------------------------------------------------------

def _build():
    SELW = NPAIR * BPC * 2 * P          # sel free size = 102400
    nc = bacc.Bacc(None, target_bir_lowering=False)

    d_sel = nc.dram_tensor("selw", [P, SELW], fp8, kind="ExternalInput")
    d_xT = nc.dram_tensor("xT", [NF, NPC], f32, kind="ExternalInput")
    d_deg = nc.dram_tensor("deg", [P, BPC], f32, kind="ExternalInput")
    d_mask = nc.dram_tensor("mask", [P, BPC], f32, kind="ExternalInput")
    d_psel = nc.dram_tensor("psel", [P, BPC * G], f32, kind="ExternalInput")
    d_W = nc.dram_tensor("W", [P, L * 2 * H], bf16, kind="ExternalInput")
    d_gb = nc.dram_tensor("gb", [1, 2 * L * H], f32, kind="ExternalInput")
    d_emb0 = nc.dram_tensor("emb0", [NF, H], f32, kind="ExternalInput")
    d_emb1 = nc.dram_tensor("emb1", [NF, H], f32, kind="ExternalInput")
    d_w1 = nc.dram_tensor("w1", [P, 2 * P], f32, kind="ExternalInput")
    d_w2 = nc.dram_tensor("w2", [P, 64], f32, kind="ExternalInput")
    d_w3 = nc.dram_tensor("w3", [64, 1], f32, kind="ExternalInput")
    d_b1 = nc.dram_tensor("b1", [P, 1], f32, kind="ExternalInput")
    d_b2 = nc.dram_tensor("b2", [64, 1], f32, kind="ExternalInput")
    d_b3 = nc.dram_tensor("b3", [1, 1], f32, kind="ExternalInput")
    d_out = nc.dram_tensor("out", [1, G], f32, kind="ExternalOutput")

    rg = [list(range(NCORE))]

    with tile.TileContext(nc) as tc, ExitStack() as ctx:
        pers = ctx.enter_context(tc.tile_pool(name="pers", bufs=1))
        psT = ctx.enter_context(tc.tile_pool(name="psT", bufs=1, space="PSUM"))
        psS = ctx.enter_context(tc.tile_pool(name="psS", bufs=1, space="PSUM"))
        psB = ctx.enter_context(tc.tile_pool(name="psB", bufs=2, space="PSUM"))
        work = ctx.enter_context(tc.tile_pool(name="work", bufs=2))
        stream = ctx.enter_context(tc.tile_pool(name="stream", bufs=2))
        dram = ctx.enter_context(tc.tile_pool(name="dram", bufs=2, space="DRAM"))

        # ---- persistent SBUF state -------------------------------------
        sel_sb = pers.tile([P, SELW], fp8, tag="sel")
        tab_sb = pers.tile([P, NBLK * H], fp8, tag="tab")
        deg_sb = pers.tile([P, BPC], f32, tag="deg")
        mask_sb = pers.tile([P, BPC], f32, tag="mask")
        W_sb = pers.tile([P, L * 2 * H], bf16, tag="W")
        gb_sb = pers.tile([1, 2 * L * H], f32, tag="gb")
        emb0_sb = pers.tile([NF, H], f32, tag="emb0")
        emb1_sb = pers.tile([NF, H], f32, tag="emb1")
        w1_sb = pers.tile([P, 2 * P], f32, tag="w1")
        w2_sb = pers.tile([P, 64], f32, tag="w2")
        w3_sb = pers.tile([64, 1], f32, tag="w3")
        b1_sb = pers.tile([P, 1], f32, tag="b1")
        b2_sb = pers.tile([64, 1], f32, tag="b2")
        b3_sb = pers.tile([1, 1], f32, tag="b3")

        h_sb = pers.tile([P, BPC * H], f32, tag="h")
        hsT_sb = pers.tile([P, BPC * 2 * P], bf16, tag="hsT")
        tq_sb = pers.tile([P, BPC * 2 * H], f32, tag="tq")   # t | t^2 per blk
        ag_sb = pers.tile([P, BPC * H], fp8, tag="ag")
        nrm_sb = pers.tile([P, BPC], f32, tag="nrm")
        nrmd_sb = pers.tile([P, BPC], f32, tag="nrmd")
        D_sb = pers.tile([NF, H], f32, tag="D")
        base_rep = pers.tile([P, H], f32, tag="base_rep")
        a_rep = pers.tile([P, H], f32, tag="a_rep")
        c_rep = pers.tile([P, H], f32, tag="c_rep")
        ident_bf = pers.tile([P, P], bf16, tag="ident")
        ones9 = pers.tile([NF, 1], f32, tag="ones9")
        ones1 = pers.tile([1, P], f32, tag="ones1")
        ones8 = pers.tile([NCORE, 1], f32, tag="ones8")
        ones128 = pers.tile([P, 1], f32, tag="ones128")
        sg_sb = pers.tile([NCORE, 2 * H], f32, tag="sg")
        stv = pers.tile([1, 2 * H], f32, tag="stv")
        scal = pers.tile([1, 8 * H], f32, tag="scal")

        # ---- DRAM bounce buffers ---------------------------------------
        ag_in = dram.tile([P, BPC * H], fp8, tag="ag_in")
        ag_outs = [dram.tile([NCORE * P, BPC * H], fp8, tag=f"ag_out{l}",
                             addr_space="Shared", name=f"ag_out{l}")
                   for l in range(L)]
        ar_in = dram.tile([1, 2 * H], f32, tag="ar_in")
        ar_outs = [dram.tile([NCORE, 2 * H], f32, tag=f"ar_out{l}",
                             addr_space="Shared", name=f"ar_out{l}")
                   for l in range(L)]
        pr_in = dram.tile([2 * P, G], f32, tag="pr_in")
        pr_out = dram.tile([2 * P, G], f32, tag="pr_out", addr_space="Shared")

        # ---- input loads (small ones first so they aren't queued behind
        # the 13 MB sel table) --------------------------------------------
        for t, d in [(deg_sb, d_deg), (mask_sb, d_mask),
                     (W_sb, d_W), (gb_sb, d_gb), (emb0_sb, d_emb0),
                     (emb1_sb, d_emb1), (w1_sb, d_w1), (w2_sb, d_w2),
                     (w3_sb, d_w3), (b1_sb, d_b1), (b2_sb, d_b2),
                     (b3_sb, d_b3)]:
            nc.sync.dma_start(out=t[:], in_=d[:])
        nc.sync.dma_start(out=sel_sb[:], in_=d_sel[:])

        make_identity(nc, ident_bf[:])
        nc.vector.memset(ones9[:], 1.0)
        nc.vector.memset(ones1[:], 1.0)
        nc.vector.memset(ones8[:], 1.0)
        nc.vector.memset(ones128[:], 1.0)

        # nrm = rsqrt(deg) * mask ; nrmd = nrm / FP8S
        rdeg = work.tile([P, BPC], f32, tag="rdeg")
        nc.vector.reciprocal(out=rdeg[:], in_=deg_sb[:])
        nc.scalar.activation(out=rdeg[:], in_=rdeg[:], func=FT.Sqrt)
        nc.vector.tensor_tensor(out=nrm_sb[:], in0=rdeg[:], in1=mask_sb[:],
                                op=OP.mult)
        nc.vector.tensor_scalar_mul(nrmd_sb[:], nrm_sb[:], 1.0 / FP8S)

        # encoder prep: D = emb1 - emb0 ; base = ones9^T @ emb0, broadcast
        nc.vector.tensor_tensor(out=D_sb[:], in0=emb1_sb[:], in1=emb0_sb[:],
                                op=OP.subtract)
        ps_b = psB.tile([1, H], f32, tag="vec")
        nc.tensor.matmul(out=ps_b[:], lhsT=ones9[:], rhs=emb0_sb[:],
                         start=True, stop=True)
        bvec = scal[:, 0:H]
        nc.vector.tensor_copy(out=bvec, in_=ps_b[:])
        ps_br = psB.tile([P, H], f32, tag="vec")
        nc.tensor.matmul(out=ps_br[:], lhsT=ones1[:], rhs=bvec,
                         start=True, stop=True)
        nc.vector.tensor_copy(out=base_rep[:], in_=ps_br[:])

        def hslice(nb):
            return h_sb[:, nb * H:(nb + 1) * H]

        def emit_hs_transpose(nb):
            """hs = h*nrm (bf16), transpose both 128-halves into hsT_sb."""
            hs_bf = work.tile([P, H], bf16, tag="hs_bf")
            nc.vector.tensor_scalar_mul(hs_bf[:], hslice(nb),
                                        nrm_sb[:, nb:nb + 1])
            for k in range(2):
                pst = psT.tile([P, P], bf16, tag=f"t{k}")
                nc.tensor.transpose(out=pst[:], in_=hs_bf[:, k * P:(k + 1) * P],
                                    identity=ident_bf[:])
                nc.vector.tensor_copy(
                    out=hsT_sb[:, (nb * 2 + k) * P:(nb * 2 + k + 1) * P],
                    in_=pst[:])

        # encoder: h0 = base + xT^T @ D  (per block)
        for nb in range(BPC):
            xT_t = stream.tile([NF, P], f32, tag="xT_t")
            nc.sync.dma_start(out=xT_t[:], in_=d_xT[:, nb * P:(nb + 1) * P])
            ps_h = psT.tile([P, H], f32, tag=f"t{nb % 5}")
            nc.tensor.matmul(out=ps_h[:], lhsT=xT_t[:],
                             rhs=D_sb[:], start=True, stop=True)
            nc.vector.tensor_tensor(out=hslice(nb), in0=ps_h[:],
                                    in1=base_rep[:], op=OP.add)
            emit_hs_transpose(nb)

        # ---- layers -----------------------------------------------------
        for l in range(L):
            # GEMM hws = hs @ W[l], cast fp8 x FP8S into ag_sb
            for nb in range(BPC):
                ps_g = psT.tile([P, H], f32, tag=f"t{nb % 5}")
                for k in range(2):
                    nc.tensor.matmul(
                        out=ps_g[:],
                        lhsT=hsT_sb[:, (nb * 2 + k) * P:(nb * 2 + k + 1) * P],
                        rhs=W_sb[:, (l * 2 + k) * H:(l * 2 + k + 1) * H],
                        start=(k == 0), stop=(k == 1))
                nc.scalar.activation(out=ag_sb[:, nb * H:(nb + 1) * H],
                                     in_=ps_g[:], func=FT.Copy, scale=FP8S)
            nc.sync.dma_start(out=ag_in[:], in_=ag_sb[:])
            nc.gpsimd.collective_compute(
                "AllGather", OP.bypass, replica_groups=rg,
                ins=[ag_in[:]], outs=[ag_outs[l][:]])
            for r in range(NCORE):
                nc.sync.dma_start(
                    out=tab_sb[:, r * BPC * H:(r + 1) * BPC * H],
                    in_=ag_outs[l][r * P:(r + 1) * P, :])

            # aggregation: two waves of 5 dst blocks over 40 src pairs
            ps_sq = psS.tile([1, 2 * H], f32, tag="sq")
            for wave in range(2):
                nbs = range(wave * 5, wave * 5 + 5)
                pts = {}
                for nb in nbs:
                    pts[nb] = psT.tile([P, H], f32, tag=f"t{nb % 5}",
                                       name=f"pt{nb}")
                for k in range(NPAIR):
                    rhs = tab_sb[:, (2 * k) * H:(2 * k + 2) * H].rearrange(
                        "p (two h) -> p two h", two=2)
                    for nb in nbs:
                        o = (k * BPC + nb) * 2 * P
                        lhsT = sel_sb[:, o:o + 2 * P].rearrange(
                            "p (two d) -> p two d", two=2)
                        nc.tensor.matmul(out=pts[nb][:], lhsT=lhsT, rhs=rhs,
                                         start=(k == 0), stop=(k == NPAIR - 1),
                                         perf_mode=DR)
                # t = nrm/FP8S * ps ; tsq = (nrm/FP8S * ps)^2 ; stats matmul
                for nb in nbs:
                    tsl = tq_sb[:, nb * 2 * H:nb * 2 * H + H]
                    sqs = tq_sb[:, nb * 2 * H + H:(nb + 1) * 2 * H]
                    nc.scalar.activation(out=tsl, in_=pts[nb][:], func=FT.Copy,
                                         scale=nrmd_sb[:, nb:nb + 1])
                    nc.scalar.activation(out=sqs, in_=pts[nb][:],
                                         func=FT.Square,
                                         scale=nrmd_sb[:, nb:nb + 1])
                    nc.tensor.matmul(
                        out=ps_sq[:],
                        lhsT=ones128[:],
                        rhs=tq_sb[:, nb * 2 * H:(nb + 1) * 2 * H],
                        start=(nb == 0), stop=(nb == BPC - 1))

            # stats: 2KB AllGather + 8-row sum
            st_sb = scal[:, 6 * H:8 * H]
            nc.vector.tensor_copy(out=st_sb, in_=ps_sq[:])
            nc.sync.dma_start(out=ar_in[:], in_=st_sb)
            nc.gpsimd.collective_compute(
                "AllGather", OP.bypass, replica_groups=rg,
                ins=[ar_in[:]], outs=[ar_outs[l][:]])
            nc.sync.dma_start(out=sg_sb[:], in_=ar_outs[l][:])
            ps_sv = psB.tile([1, 2 * H], f32, tag="vec")
            nc.tensor.matmul(out=ps_sv[:], lhsT=ones8[:], rhs=sg_sb[:],
                             start=True, stop=True)
            nc.vector.tensor_copy(out=stv[:], in_=ps_sv[:])

            # a = gamma*istd ; c = beta - mu*a   (all [1,H] lanes)
            mu = scal[:, H:2 * H]
            var = scal[:, 2 * H:3 * H]
            av = scal[:, 3 * H:4 * H]
            cv = scal[:, 4 * H:5 * H]
            msq = scal[:, 5 * H:6 * H]
            nc.vector.tensor_scalar_mul(mu, stv[:, 0:H], 1.0 / N)
            nc.vector.tensor_scalar_mul(var, stv[:, H:2 * H], 1.0 / N)
            nc.vector.tensor_tensor(out=msq, in0=mu, in1=mu, op=OP.mult)
            nc.vector.tensor_tensor(out=var, in0=var, in1=msq, op=OP.subtract)
            nc.vector.tensor_scalar_add(var, var, BN_EPS)
            nc.vector.reciprocal(out=var, in_=var)
            nc.scalar.activation(out=var, in_=var, func=FT.Sqrt)  # istd
            nc.vector.tensor_tensor(out=av, in0=var,
                                    in1=gb_sb[:, l * H:(l + 1) * H], op=OP.mult)
            nc.vector.tensor_tensor(out=msq, in0=mu, in1=av, op=OP.mult)
            nc.vector.tensor_tensor(out=cv, in0=gb_sb[:, (L + l) * H:(L + l + 1) * H],
                                    in1=msq, op=OP.subtract)
            ps_a = psB.tile([P, H], f32, tag="vec")
            nc.tensor.matmul(out=ps_a[:], lhsT=ones1[:], rhs=av,
                             start=True, stop=True)
            nc.vector.tensor_copy(out=a_rep[:], in_=ps_a[:])
            ps_c = psB.tile([P, H], f32, tag="vec")
            nc.tensor.matmul(out=ps_c[:], lhsT=ones1[:], rhs=cv,
                             start=True, stop=True)
            nc.vector.tensor_copy(out=c_rep[:], in_=ps_c[:])

            # h = relu(t*a + c) + h ; prepare hsT for next layer
            for nb in range(BPC):
                tsl = tq_sb[:, nb * 2 * H:nb * 2 * H + H]
                u = tq_sb[:, nb * 2 * H + H:(nb + 1) * 2 * H]  # reuse sq slot
                nc.vector.tensor_tensor(out=u, in0=tsl, in1=a_rep[:], op=OP.mult)
                nc.vector.tensor_tensor(out=u, in0=u, in1=c_rep[:], op=OP.add)
                nc.vector.scalar_tensor_tensor(
                    out=hslice(nb), in0=u, scalar=0.0, in1=hslice(nb),
                    op0=OP.max, op1=OP.add)
                if l < L - 1:
                    emit_hs_transpose(nb)

        # ---- pooling ----------------------------------------------------
        ps_p0 = psT.tile([P, G], f32, tag="t0")
        ps_p1 = psT.tile([P, G], f32, tag="t1")
        for nb in range(BPC):
            psel_t = stream.tile([P, G], f32, tag="psel_t")
            nc.sync.dma_start(out=psel_t[:], in_=d_psel[:, nb * G:(nb + 1) * G])
            nc.tensor.matmul(out=ps_p0[:], lhsT=h_sb[:, nb * H:nb * H + P],
                             rhs=psel_t[:], start=(nb == 0), stop=(nb == BPC - 1))
            nc.tensor.matmul(out=ps_p1[:], lhsT=h_sb[:, nb * H + P:(nb + 1) * H],
                             rhs=psel_t[:], start=(nb == 0), stop=(nb == BPC - 1))
        g0 = work.tile([P, G], f32, tag="g0")
        g1 = work.tile([P, G], f32, tag="g1")
        nc.vector.tensor_copy(out=g0[:], in_=ps_p0[:])
        nc.vector.tensor_copy(out=g1[:], in_=ps_p1[:])
        nc.sync.dma_start(out=pr_in[0:P, :], in_=g0[:])
        nc.sync.dma_start(out=pr_in[P:2 * P, :], in_=g1[:])
        nc.gpsimd.collective_compute(
            "AllReduce", OP.add, replica_groups=rg,
            ins=[pr_in[:]], outs=[pr_out[:]])
        nc.sync.dma_start(out=g0[:], in_=pr_out[0:P, :])
        nc.sync.dma_start(out=g1[:], in_=pr_out[P:2 * P, :])

        # MLP head (transposed: weights are lhsT, graphs along free dim)
        ps1 = psB.tile([P, G], f32, tag="vec")
        nc.tensor.matmul(out=ps1[:], lhsT=w1_sb[:, 0:P], rhs=g0[:],
                         start=True, stop=False)
        nc.tensor.matmul(out=ps1[:], lhsT=w1_sb[:, P:2 * P], rhs=g1[:],
                         start=False, stop=True)
        y1 = work.tile([P, G], f32, tag="y1")
        nc.scalar.activation(out=y1[:], in_=ps1[:], func=FT.Relu,
                             bias=b1_sb[:, 0:1])
        ps2 = psB.tile([64, G], f32, tag="vec")
        nc.tensor.matmul(out=ps2[:], lhsT=w2_sb[:], rhs=y1[:],
                         start=True, stop=True)
        y2 = work.tile([64, G], f32, tag="y2")
        nc.scalar.activation(out=y2[:], in_=ps2[:], func=FT.Relu,
                             bias=b2_sb[:, 0:1])
        ps3 = psB.tile([1, G], f32, tag="vec")
        nc.tensor.matmul(out=ps3[:], lhsT=w3_sb[:], rhs=y2[:],
                         start=True, stop=True)
        y3 = work.tile([1, G], f32, tag="y3")
        nc.vector.tensor_scalar_add(y3[:], ps3[:], b3_sb[0:1, 0:1])
        nc.sync.dma_start(out=d_out[:], in_=y3[:])

    nc.compile()
    return nc


# --------------------------------------------------------------------------
# entry point
# --------------------------------------------------------------------------

def kernel(x, edge_index, batch_ids, emb, W, b, gamma, beta,
           mlp_W1, mlp_b1, mlp_W2, mlp_b2, mlp_W3, mlp_b3,
           _trace=False, _trace_kwargs=None):
    # NB: reference BN subtracts the per-channel mean, so the additive bias b
    # cancels exactly and is not needed by the device program.
    in_maps = _preprocess(x, edge_index, batch_ids, emb, W, gamma, beta,
                          mlp_W1, mlp_b1, mlp_W2, mlp_b2, mlp_W3, mlp_b3)
    if "nc" not in _compiled:
        _compiled["nc"] = _build()
    nc = _compiled["nc"]
    kw = {}
    if _trace:
        kw = dict(trace=True, **(_trace_kwargs or {}))
    res = run_bass_kernel_spmd(nc, in_maps, core_ids=list(range(NCORE)), **kw)
    out = np.asarray(res.results[0]["out"], np.float32).reshape(G, 1)
    kernel._last_results = res
    return out


# revision 17
# speedup vs baseline: 3.9737x; 1.1797x over previous
"""Trainium2 Bass kernel for HIVNet GCN message passing (8-core SPMD).

Strategy (v3 — transposed dense aggregation, table-stationary):
  - Pad N=10000 nodes to 10240 = 80 blocks x 128; core c owns 10 dst-blocks
    (1280 nodes).  Node state h is kept TRANSPOSED: hT[half][h, n] with the
    hidden dim on partitions (2 halves of 128) and the core's 1280 nodes on
    the free axis.
  - Per layer: GEMM hws = hs @ W[l] produces node-major [128, 256] blocks
    (lhsT = hsT directly, no transposes), cast to fp8e4m3 (x32), AllGather'd
    into a DRAM table; remote shards are loaded into SBUF.
  - Aggregation (TensorE, DoubleRow fp8): stationary = table block-pair
    [128, 2, 128-H-half], moving = host-built dense edge-count matrix
    sel[p, i, dst] over the core's 1280 dst in 512-wide chunks.  psHT[half]
    [128, 1280] accumulates over all 40 pairs; the core's OWN 5 pairs read
    the local fp8 copy and overlap the AllGather of the rest.
  - BN: fused DVE tensor_tensor_reduce produces t = ps*nrm/S (+sum) and
    t^2 (+sumsq) in two passes; [128, 4] AllReduce; a,c are per-partition so
    apply is a single fused ACT Relu(a*t + c) per half + residual add.
  - Readout: transpose h once, graph mean-pool one-hots (1/cnt folded),
    2*128-row AllReduce, 3-layer MLP.
"""

import sys

sys.path.insert(0, "/opt/trn_rl_repo")

from contextlib import ExitStack

import numpy as np
import ml_dtypes

from concourse import bass, mybir, bacc, tile
from concourse.bass_utils import run_bass_kernel_spmd
from concourse.masks import make_identity

NCORE = 8
P = 128
H = 256
L = 4
NF = 9
G = 256
N = 10000
BPC = 10                # dst blocks per core
NPC = BPC * P           # 1280 nodes per core
NPAD = NCORE * NPC      # 10240
NBLK = NPAD // P        # 80 src blocks
NPAIR = NBLK // 2       # 40 src block pairs (DoubleRow K=256)
BN_EPS = 1e-5
FP8S = 32.0             # fp8 table scale
CHUNKS = [(0, 512), (512, 512), (1024, 256)]   # dst chunks (<=512 f32 PSUM bank)

f32 = mybir.dt.float32
bf16 = mybir.dt.bfloat16
fp8 = mybir.dt.float8e4
fp8np = mybir.dt.np(mybir.dt.float8e4)
bfnp = ml_dtypes.bfloat16

FT = mybir.ActivationFunctionType
OP = mybir.AluOpType
DRM = mybir.MatmulPerfMode.DoubleRow

_compiled = {}


# --------------------------------------------------------------------------
# host-side structural preprocessing
# --------------------------------------------------------------------------

def _preprocess(x, edge_index, batch_ids, emb, W, gamma, beta,
                mlp_W1, mlp_b1, mlp_W2, mlp_b2, mlp_W3, mlp_b3):
    src = np.asarray(edge_index[0], np.int64)
    dst = np.asarray(edge_index[1], np.int64)
    src_all = np.concatenate([src, np.arange(N, dtype=np.int64)])
    dst_all = np.concatenate([dst, np.arange(N, dtype=np.int64)])

    deg = np.bincount(dst_all, minlength=NPAD).astype(np.float64)
    nrm_full = np.zeros(NPAD, np.float32)
    nrm_full[:NPAD] = 1.0 / np.sqrt(np.maximum(deg, 1.0))
    nrm_full[deg == 0] = 0.0

    # dense per-core sel (moving operand): [128 p, 40 pair, 2 i, 1280 dst]
    p_idx = (src_all % P).astype(np.int64)
    blk = src_all // P
    k_idx = blk // 2
    i_idx = blk % 2
    core = dst_all // NPC
    d_idx = dst_all % NPC
    sels = []
    for c in range(NCORE):
        m = core == c
        selc = np.zeros((P, NPAIR, 2, NPC), np.float32)
        np.add.at(selc, (p_idx[m], k_idx[m], i_idx[m], d_idx[m]), 1.0)
        sels.append(selc.reshape(P, NPAIR * 2 * NPC).astype(fp8np))

    # graph pool one-hot with 1/count folded in
    bids = np.asarray(batch_ids, np.int64)
    cnt = np.bincount(bids, minlength=G).astype(np.float32)
    inv = 1.0 / np.maximum(cnt, 1.0)
    psel_full = np.zeros((NPAD, G), np.float32)
    psel_full[np.arange(N), bids] = inv[bids]

    x_np = np.zeros((NPAD, NF), np.float32)
    x_np[:N] = np.asarray(x, np.float64)

    # encoder prep on host: D = emb1 - emb0, base = sum_f emb0[f]
    embf = np.asarray(emb, np.float32)
    D = np.ascontiguousarray(embf[:, 1, :] - embf[:, 0, :])       # [9, 256]
    baseT = np.ascontiguousarray(embf[:, 0, :].sum(0).reshape(2, P).T)  # [128,2]

    Wf = np.asarray(W, np.float32)
    W_lhsT = Wf.reshape(L, 2, P, H).transpose(2, 0, 1, 3).reshape(P, L * 2 * H)
    # gamma/beta transposed per half: [128, L*4] = (g0,g1,b0,b1) per layer
    gaT = np.asarray(gamma, np.float32).reshape(L, 2, P)
    beT = np.asarray(beta, np.float32).reshape(L, 2, P)
    gbT = np.concatenate([gaT, beT], axis=1).transpose(2, 0, 1).reshape(P, L * 4)

    w1 = np.asarray(mlp_W1, np.float32).reshape(2, P, P).transpose(1, 0, 2).reshape(P, 2 * P)
    w2 = np.asarray(mlp_W2, np.float32)
    w3 = np.asarray(mlp_W3, np.float32)
    b1 = np.asarray(mlp_b1, np.float32).reshape(P, 1)
    b2 = np.asarray(mlp_b2, np.float32).reshape(64, 1)
    b3 = np.asarray(mlp_b3, np.float32).reshape(1, 1)

    in_maps = []
    for c in range(NCORE):
        lo, hi = c * NPC, (c + 1) * NPC
        nrmc = nrm_full[lo:hi]
        nrm_exp = np.broadcast_to(nrmc, (P, NPC)).copy()          # [128,1280]
        nrms_exp = (nrm_exp / FP8S).astype(np.float32)

        pselc = psel_full[lo:hi].reshape(BPC, P, G)
        pselc = np.ascontiguousarray(pselc.transpose(1, 0, 2)).reshape(P, BPC * G)

        xTc = np.ascontiguousarray(x_np[lo:hi].T)                 # [9, 1280]

        in_maps.append(dict(
            selw=sels[c], xT=xTc, psel=pselc,
            nrme=nrm_exp, nrmse=nrms_exp,
            W=W_lhsT.astype(bfnp), gbT=gbT, D=D, baseT=baseT,
            w1=w1, w2=w2, w3=w3, b1=b1, b2=b2, b3=b3,
        ))
    return in_maps


# --------------------------------------------------------------------------
# device program
# --------------------------------------------------------------------------

def _build():
    SELW = NPAIR * 2 * NPC          # 102400
    nc = bacc.Bacc(None, target_bir_lowering=False)

    d_sel = nc.dram_tensor("selw", [P, SELW], fp8, kind="ExternalInput")
    d_xT = nc.dram_tensor("xT", [NF, NPC], f32, kind="ExternalInput")
    d_psel = nc.dram_tensor("psel", [P, BPC * G], f32, kind="ExternalInput")
    d_nrme = nc.dram_tensor("nrme", [P, NPC], f32, kind="ExternalInput")
    d_nrmse = nc.dram_tensor("nrmse", [P, NPC], f32, kind="ExternalInput")
    d_W = nc.dram_tensor("W", [P, L * 2 * H], bf16, kind="ExternalInput")
    d_gbT = nc.dram_tensor("gbT", [P, L * 4], f32, kind="ExternalInput")
    d_D = nc.dram_tensor("D", [NF, H], f32, kind="ExternalInput")
    d_baseT = nc.dram_tensor("baseT", [P, 2], f32, kind="ExternalInput")
    d_w1 = nc.dram_tensor("w1", [P, 2 * P], f32, kind="ExternalInput")
    d_w2 = nc.dram_tensor("w2", [P, 64], f32, kind="ExternalInput")
    d_w3 = nc.dram_tensor("w3", [64, 1], f32, kind="ExternalInput")
    d_b1 = nc.dram_tensor("b1", [P, 1], f32, kind="ExternalInput")
    d_b2 = nc.dram_tensor("b2", [64, 1], f32, kind="ExternalInput")
    d_b3 = nc.dram_tensor("b3", [1, 1], f32, kind="ExternalInput")
    d_out = nc.dram_tensor("out", [1, G], f32, kind="ExternalOutput")

    rg = [list(range(NCORE))]

    with tile.TileContext(nc) as tc, ExitStack() as ctx:
        pers = ctx.enter_context(tc.tile_pool(name="pers", bufs=1))
        psH = ctx.enter_context(tc.tile_pool(name="psH", bufs=1, space="PSUM"))
        psB = ctx.enter_context(tc.tile_pool(name="psB", bufs=1, space="PSUM"))
        work = ctx.enter_context(tc.tile_pool(name="work", bufs=2))
        stream = ctx.enter_context(tc.tile_pool(name="stream", bufs=2))
        dram = ctx.enter_context(tc.tile_pool(name="dram", bufs=2, space="DRAM"))

        # ---- persistent SBUF state -------------------------------------
        sel_sb = pers.tile([P, SELW], fp8, tag="sel")
        tab_sb = pers.tile([P, NBLK * H], fp8, tag="tab")
        nrme_sb = pers.tile([P, NPC], f32, tag="nrme")
        nrmse_sb = pers.tile([P, NPC], f32, tag="nrmse")
        W_sb = pers.tile([P, L * 2 * H], bf16, tag="W")
        gbT_sb = pers.tile([P, L * 4], f32, tag="gbT")
        D_sb = pers.tile([NF, H], f32, tag="D")
        baseT_sb = pers.tile([P, 2], f32, tag="baseT")
        w1_sb = pers.tile([P, 2 * P], f32, tag="w1")
        w2_sb = pers.tile([P, 64], f32, tag="w2")
        w3_sb = pers.tile([64, 1], f32, tag="w3")
        b1_sb = pers.tile([P, 1], f32, tag="b1")
        b2_sb = pers.tile([64, 1], f32, tag="b2")
        b3_sb = pers.tile([1, 1], f32, tag="b3")

        hT_sb = pers.tile([P, 2 * NPC], f32, tag="hT")      # halves side by side
        tT_sb = pers.tile([P, 2 * NPC], f32, tag="tT")
        sq_sb = pers.tile([P, NPC], f32, tag="sqs")         # scratch for t^2
        hsT_sb = pers.tile([P, 2 * NPC], bf16, tag="hsT")
        ag_sb = pers.tile([P, BPC * H], fp8, tag="ag")
        stat_sb = pers.tile([P, 4], f32, tag="stat")
        ac_sb = pers.tile([P, 8], f32, tag="ac")            # mu0 mu1 a0 a1 c0 c1 tmp
        ident_f = pers.tile([P, P], f32, tag="ident")

        # ---- DRAM bounce buffers ---------------------------------------
        ag_in = dram.tile([P, BPC * H], fp8, tag="ag_in")
        ag_outs = [dram.tile([NCORE * P, BPC * H], fp8, tag=f"ag_out{l}",
                             addr_space="Shared", name=f"ag_out{l}")
                   for l in range(L)]
        ar_in = dram.tile([P, 4], f32, tag="ar_in")
        ar_outs = [dram.tile([P, 4], f32, tag=f"ar_out{l}",
                             addr_space="Shared", name=f"ar_out{l}")
                   for l in range(L)]
        pr_in = dram.tile([2 * P, G], f32, tag="pr_in")
        pr_out = dram.tile([2 * P, G], f32, tag="pr_out", addr_space="Shared")

        # ---- input loads (small first; 13MB sel last) -------------------
        for t, d in [(nrme_sb, d_nrme), (nrmse_sb, d_nrmse), (W_sb, d_W),
                     (gbT_sb, d_gbT), (D_sb, d_D), (baseT_sb, d_baseT),
                     (w1_sb, d_w1), (w2_sb, d_w2), (w3_sb, d_w3),
                     (b1_sb, d_b1), (b2_sb, d_b2), (b3_sb, d_b3)]:
            nc.sync.dma_start(out=t[:], in_=d[:])
        nc.sync.dma_start(out=sel_sb[:], in_=d_sel[:])
        make_identity(nc, ident_f[:])

        def hT(half):
            return hT_sb[:, half * NPC:(half + 1) * NPC]

        def tT(half):
            return tT_sb[:, half * NPC:(half + 1) * NPC]

        def hsT(half):
            return hsT_sb[:, half * NPC:(half + 1) * NPC]

        # ---- encoder: hT = D^T @ xT + baseT -----------------------------
        psHT = [psH.tile([P, NPC], f32, tag=f"h{i}", name=f"psHT{i}")
                for i in range(2)]
        xT_sb = stream.tile([NF, NPC], f32, tag="xT_sb")
        nc.sync.dma_start(out=xT_sb[:], in_=d_xT[:])
        for half in range(2):
            for (off, ln) in CHUNKS:
                nc.tensor.matmul(out=psHT[half][:, off:off + ln],
                                 lhsT=D_sb[:, half * P:(half + 1) * P],
                                 rhs=xT_sb[:, off:off + ln],
                                 start=True, stop=True)
            nc.vector.tensor_scalar_add(hT(half), psHT[half][:],
                                        baseT_sb[:, half:half + 1])
            # hs = h * nrm (bf16 for the GEMM)
            nc.vector.tensor_tensor(out=hsT(half), in0=hT(half),
                                    in1=nrme_sb[:], op=OP.mult)

        # ---- layers -----------------------------------------------------
        for l in range(L):
            # GEMM hws = hs @ W[l] per dst block, cast to fp8 table shard
            for nb in range(BPC):
                ps_g = psB.tile([P, H], f32, tag="mm" if nb % 2 == 0 else "mm2",
                                name=f"ps_g{nb}")
                for half in range(2):
                    nc.tensor.matmul(
                        out=ps_g[:],
                        lhsT=hsT(half)[:, nb * P:(nb + 1) * P],
                        rhs=W_sb[:, (l * 2 + half) * H:(l * 2 + half + 1) * H],
                        start=(half == 0), stop=(half == 1))
                nc.scalar.activation(out=ag_sb[:, nb * H:(nb + 1) * H],
                                     in_=ps_g[:], func=FT.Copy, scale=FP8S)
            nc.sync.dma_start(out=ag_in[:], in_=ag_sb[:])
            nc.gpsimd.collective_compute(
                "AllGather", OP.bypass, replica_groups=rg,
                ins=[ag_in[:]], outs=[ag_outs[l][:]])
            for r in range(NCORE):
                nc.sync.dma_start(
                    out=tab_sb[:, r * BPC * H:(r + 1) * BPC * H],
                    in_=ag_outs[l][r * P:(r + 1) * P, :])

            # aggregation: psHT[half] += tab_pair^T (DR) @ sel chunks
            psHT = [psH.tile([P, NPC], f32, tag=f"h{i}", name=f"psT{l}{i}")
                    for i in range(2)]
            for k in range(NPAIR):
                # pair region views: tab [128, 2 blk, 256 H], sel [128, 2, 1280]
                tpair = tab_sb[:, 2 * k * H:(2 * k + 2) * H].rearrange(
                    "p (two h) -> p two h", two=2)
                spair = sel_sb[:, k * 2 * NPC:(k + 1) * 2 * NPC].rearrange(
                    "p (two d) -> p two d", two=2)
                for half in range(2):
                    lhsT = tpair[:, :, half * P:(half + 1) * P]
                    for (off, ln) in CHUNKS:
                        nc.tensor.matmul(
                            out=psHT[half][:, off:off + ln],
                            lhsT=lhsT,
                            rhs=spair[:, :, off:off + ln],
                            start=(k == 0), stop=(k == NPAIR - 1),
                            perf_mode=DRM)

            # t = ps*nrm/S (+col-sum), sq = t*t (+col-sumsq)  — fused DVE
            for half in range(2):
                nc.vector.scalar_tensor_tensor(
                    out=tT(half), in0=psHT[half][:], scalar=1.0,
                    in1=nrmse_sb[:], op0=OP.mult, op1=OP.mult,
                    accum_out=stat_sb[:, half:half + 1])
                nc.vector.scalar_tensor_tensor(
                    out=sq_sb[:], in0=tT(half), scalar=1.0,
                    in1=tT(half), op0=OP.mult, op1=OP.mult,
                    accum_out=stat_sb[:, 2 + half:3 + half])
            nc.sync.dma_start(out=ar_in[:], in_=stat_sb[:])
            nc.gpsimd.collective_compute(
                "AllReduce", OP.add, replica_groups=rg,
                ins=[ar_in[:]], outs=[ar_outs[l][:]])
            nc.sync.dma_start(out=stat_sb[:], in_=ar_outs[l][:])

            # per-partition BN coeffs: a = gamma*istd, c = beta - mu*a
            mu2 = ac_sb[:, 0:2]
            var2 = ac_sb[:, 2:4]
            a2 = ac_sb[:, 4:6]
            c2 = ac_sb[:, 6:8]
            nc.vector.tensor_scalar_mul(mu2, stat_sb[:, 0:2], 1.0 / N)
            nc.vector.tensor_scalar_mul(var2, stat_sb[:, 2:4], 1.0 / N)
            nc.vector.tensor_tensor(out=a2, in0=mu2, in1=mu2, op=OP.mult)
            nc.vector.tensor_tensor(out=var2, in0=var2, in1=a2, op=OP.subtract)
            nc.vector.tensor_scalar_add(var2, var2, BN_EPS)
            nc.vector.reciprocal(out=var2, in_=var2)
            nc.scalar.activation(out=var2, in_=var2, func=FT.Sqrt)  # istd
            nc.vector.tensor_tensor(out=a2, in0=var2,
                                    in1=gbT_sb[:, l * 4:l * 4 + 2], op=OP.mult)
            nc.vector.tensor_tensor(out=c2, in0=mu2, in1=a2, op=OP.mult)
            nc.vector.tensor_tensor(out=c2, in0=gbT_sb[:, l * 4 + 2:l * 4 + 4],
                                    in1=c2, op=OP.subtract)

            # h += relu(a*t + c) ; hs = h*nrm for next GEMM
            for half in range(2):
                r_t = work.tile([P, NPC], f32, tag="r_t")
                nc.scalar.activation(out=r_t[:], in_=tT(half), func=FT.Relu,
                                     scale=ac_sb[:, 4 + half:5 + half],
                                     bias=ac_sb[:, 6 + half:7 + half])
                nc.vector.tensor_tensor(out=hT(half), in0=hT(half),
                                        in1=r_t[:], op=OP.add)
                if l < L - 1:
                    nc.vector.tensor_tensor(out=hsT(half), in0=hT(half),
                                            in1=nrme_sb[:], op=OP.mult)

        # ---- pooling: transpose hT blocks, one-hot matmul ---------------
        ps_p0 = psB.tile([P, G], f32, tag="mm")
        ps_p1 = psB.tile([P, G], f32, tag="mm2")
        hblk = [work.tile([P, P], f32, tag=f"hp{i}", name=f"hblk{i}")
                for i in range(2)]
        for nb in range(BPC):
            psel_t = stream.tile([P, G], f32, tag="psel_t")
            nc.sync.dma_start(out=psel_t[:], in_=d_psel[:, nb * G:(nb + 1) * G])
            for half in range(2):
                ps_tr = psH.tile([P, P], f32, tag=f"h{half}", name=f"ptr{half}")
                nc.tensor.transpose(out=ps_tr[:],
                                    in_=hT(half)[:, nb * P:(nb + 1) * P],
                                    identity=ident_f[:])
                nc.vector.tensor_copy(out=hblk[half][:], in_=ps_tr[:])
            nc.tensor.matmul(out=ps_p0[:], lhsT=hblk[0][:], rhs=psel_t[:],
                             start=(nb == 0), stop=(nb == BPC - 1))
            nc.tensor.matmul(out=ps_p1[:], lhsT=hblk[1][:], rhs=psel_t[:],
                             start=(nb == 0), stop=(nb == BPC - 1))
        g0 = work.tile([P, G], f32, tag="g0")
        g1 = work.tile([P, G], f32, tag="g1")
        nc.vector.tensor_copy(out=g0[:], in_=ps_p0[:])
        nc.vector.tensor_copy(out=g1[:], in_=ps_p1[:])
        nc.sync.dma_start(out=pr_in[0:P, :], in_=g0[:])
        nc.sync.dma_start(out=pr_in[P:2 * P, :], in_=g1[:])
        nc.gpsimd.collective_compute(
            "AllReduce", OP.add, replica_groups=rg,
            ins=[pr_in[:]], outs=[pr_out[:]])
        nc.sync.dma_start(out=g0[:], in_=pr_out[0:P, :])
        nc.sync.dma_start(out=g1[:], in_=pr_out[P:2 * P, :])

        # MLP head (weights as lhsT, graphs along free dim)
        ps1 = psB.tile([P, G], f32, tag="mm")
        nc.tensor.matmul(out=ps1[:], lhsT=w1_sb[:, 0:P], rhs=g0[:],
                         start=True, stop=False)
        nc.tensor.matmul(out=ps1[:], lhsT=w1_sb[:, P:2 * P], rhs=g1[:],
                         start=False, stop=True)
        y1 = work.tile([P, G], f32, tag="y1")
        nc.scalar.activation(out=y1[:], in_=ps1[:], func=FT.Relu,
                             bias=b1_sb[:, 0:1])
        ps2 = psB.tile([64, G], f32, tag="mm2")
        nc.tensor.matmul(out=ps2[:], lhsT=w2_sb[:], rhs=y1[:],
                         start=True, stop=True)
        y2 = work.tile([64, G], f32, tag="y2")
        nc.scalar.activation(out=y2[:], in_=ps2[:], func=FT.Relu,
                             bias=b2_sb[:, 0:1])
        ps3 = psB.tile([1, G], f32, tag="mm")
        nc.tensor.matmul(out=ps3[:], lhsT=w3_sb[:], rhs=y2[:],
                         start=True, stop=True)
        y3 = work.tile([1, G], f32, tag="y3")
        nc.vector.tensor_scalar_add(y3[:], ps3[:], b3_sb[0:1, 0:1])
        nc.sync.dma_start(out=d_out[:], in_=y3[:])

    nc.compile()
    return nc


# --------------------------------------------------------------------------
# entry point
# --------------------------------------------------------------------------

def kernel(x, edge_index, batch_ids, emb, W, b, gamma, beta,
           mlp_W1, mlp_b1, mlp_W2, mlp_b2, mlp_W3, mlp_b3,
           _trace=False, _trace_kwargs=None):
    in_maps = _preprocess(x, edge_index, batch_ids, emb, W, gamma, beta,
                          mlp_W1, mlp_b1, mlp_W2, mlp_b2, mlp_W3, mlp_b3)
    if "nc" not in _compiled:
        _compiled["nc"] = _build()
    nc = _compiled["nc"]
    kw = {}
    if _trace:
        kw = dict(trace=True, **(_trace_kwargs or {}))
    res = run_bass_kernel_spmd(nc, in_maps, core_ids=list(range(NCORE)), **kw)
    out = np.asarray(res.results[0]["out"], np.float32).reshape(G, 1)
    kernel._last_results = res
    return out


# revision 22
# speedup vs baseline: 4.3074x; 1.0840x over previous
"""Trainium2 Bass kernel for HIVNet GCN message passing (8-core SPMD).

Strategy (v3 — transposed dense aggregation, table-stationary):
  - Pad N=10000 nodes to 10240 = 80 blocks x 128; core c owns 10 dst-blocks
    (1280 nodes).  Node state h is kept TRANSPOSED: hT[half][h, n] with the
    hidden dim on partitions (2 halves of 128) and the core's 1280 nodes on
    the free axis.
  - Per layer: GEMM hws = hs @ W[l] produces node-major [128, 256] blocks
    (lhsT = hsT directly, no transposes), cast to fp8e4m3 (x32), AllGather'd
    into a DRAM table; remote shards are loaded into SBUF.
  - Aggregation (TensorE, DoubleRow fp8): stationary = table block-pair
    [128, 2, 128-H-half], moving = host-built dense edge-count matrix
    sel[p, i, dst] over the core's 1280 dst in 512-wide chunks.  psHT[half]
    [128, 1280] accumulates over all 40 pairs; the core's OWN 5 pairs read
    the local fp8 copy and overlap the AllGather of the rest.
  - BN: fused DVE tensor_tensor_reduce produces t = ps*nrm/S (+sum) and
    t^2 (+sumsq) in two passes; [128, 4] AllReduce; a,c are per-partition so
    apply is a single fused ACT Relu(a*t + c) per half + residual add.
  - Readout: transpose h once, graph mean-pool one-hots (1/cnt folded),
    2*128-row AllReduce, 3-layer MLP.
"""

import sys

sys.path.insert(0, "/opt/trn_rl_repo")

from contextlib import ExitStack

import numpy as np
import ml_dtypes

from concourse import bass, mybir, bacc, tile
from concourse.bass_utils import run_bass_kernel_spmd
from concourse.masks import make_identity

NCORE = 8
P = 128
H = 256
L = 4
NF = 9
G = 256
N = 10000
BPC = 10                # dst blocks per core
NPC = BPC * P           # 1280 nodes per core
NPAD = NCORE * NPC      # 10240
NBLK = NPAD // P        # 80 src blocks
NPAIR = NBLK // 2       # 40 src block pairs (DoubleRow K=256)
BN_EPS = 1e-5
FP8S = 32.0             # fp8 table scale
CHUNKS = [(0, 512), (512, 512), (1024, 256)]   # dst chunks (<=512 f32 PSUM bank)

f32 = mybir.dt.float32
bf16 = mybir.dt.bfloat16
fp8 = mybir.dt.float8e4
fp8np = mybir.dt.np(mybir.dt.float8e4)
bfnp = ml_dtypes.bfloat16

FT = mybir.ActivationFunctionType
OP = mybir.AluOpType
DRM = mybir.MatmulPerfMode.DoubleRow

_compiled = {}


# --------------------------------------------------------------------------
# host-side structural preprocessing
# --------------------------------------------------------------------------

def _preprocess(x, edge_index, batch_ids, emb, W, gamma, beta,
                mlp_W1, mlp_b1, mlp_W2, mlp_b2, mlp_W3, mlp_b3):
    src = np.asarray(edge_index[0], np.int64)
    dst = np.asarray(edge_index[1], np.int64)
    src_all = np.concatenate([src, np.arange(N, dtype=np.int64)])
    dst_all = np.concatenate([dst, np.arange(N, dtype=np.int64)])

    deg = np.bincount(dst_all, minlength=NPAD).astype(np.float64)
    nrm_full = np.zeros(NPAD, np.float32)
    nrm_full[:NPAD] = 1.0 / np.sqrt(np.maximum(deg, 1.0))
    nrm_full[deg == 0] = 0.0

    # dense per-core sel (moving operand): [128 p, 40 pair, 2 i, 1280 dst]
    p_idx = (src_all % P).astype(np.int64)
    blk = src_all // P
    k_idx = blk // 2
    i_idx = blk % 2
    core = dst_all // NPC
    d_idx = dst_all % NPC
    sels = []
    for c in range(NCORE):
        m = core == c
        selc = np.zeros((P, NPAIR, 2, NPC), np.float32)
        np.add.at(selc, (p_idx[m], k_idx[m], i_idx[m], d_idx[m]), 1.0)
        sels.append(selc.reshape(P, NPAIR * 2 * NPC).astype(fp8np))

    # graph pool one-hot with 1/count folded in
    bids = np.asarray(batch_ids, np.int64)
    cnt = np.bincount(bids, minlength=G).astype(np.float32)
    inv = 1.0 / np.maximum(cnt, 1.0)
    psel_full = np.zeros((NPAD, G), np.float32)
    psel_full[np.arange(N), bids] = inv[bids]

    x_np = np.zeros((NPAD, NF), np.float32)
    x_np[:N] = np.asarray(x, np.float64)

    # encoder prep on host: D = emb1 - emb0, base = sum_f emb0[f]
    embf = np.asarray(emb, np.float32)
    D = np.ascontiguousarray(embf[:, 1, :] - embf[:, 0, :])       # [9, 256]
    baseT = np.ascontiguousarray(embf[:, 0, :].sum(0).reshape(2, P).T)  # [128,2]

    Wf = np.asarray(W, np.float32)
    W_lhsT = Wf.reshape(L, 2, P, H).transpose(2, 0, 1, 3).reshape(P, L * 2 * H)
    # gamma/beta transposed per half: [128, L*4] = (g0,g1,b0,b1) per layer
    gaT = np.asarray(gamma, np.float32).reshape(L, 2, P)
    beT = np.asarray(beta, np.float32).reshape(L, 2, P)
    gbT = np.concatenate([gaT, beT], axis=1).transpose(2, 0, 1).reshape(P, L * 4)

    w1 = np.asarray(mlp_W1, np.float32).reshape(2, P, P).transpose(1, 0, 2).reshape(P, 2 * P)
    w2 = np.asarray(mlp_W2, np.float32)
    w3 = np.asarray(mlp_W3, np.float32)
    b1 = np.asarray(mlp_b1, np.float32).reshape(P, 1)
    b2 = np.asarray(mlp_b2, np.float32).reshape(64, 1)
    b3 = np.asarray(mlp_b3, np.float32).reshape(1, 1)

    in_maps = []
    for c in range(NCORE):
        lo, hi = c * NPC, (c + 1) * NPC
        nrmc = nrm_full[lo:hi]
        nrm_exp = np.broadcast_to(nrmc, (P, NPC)).copy()          # [128,1280]
        nrms_exp = (nrm_exp / FP8S).astype(np.float32)

        pselc = psel_full[lo:hi].reshape(BPC, P, G)
        pselc = np.ascontiguousarray(pselc.transpose(1, 0, 2)).reshape(P, BPC * G)

        xTc = np.ascontiguousarray(x_np[lo:hi].T)                 # [9, 1280]

        in_maps.append(dict(
            selw=sels[c], xT=xTc, psel=pselc,
            nrme=nrm_exp, nrmse=nrms_exp,
            W=W_lhsT.astype(bfnp), gbT=gbT, D=D, baseT=baseT,
            w1=w1, w2=w2, w3=w3, b1=b1, b2=b2, b3=b3,
        ))
    return in_maps


# --------------------------------------------------------------------------
# device program
# --------------------------------------------------------------------------

def _build():
    SELW = NPAIR * 2 * NPC          # 102400
    nc = bacc.Bacc(None, target_bir_lowering=False)

    d_sel = nc.dram_tensor("selw", [P, SELW], fp8, kind="ExternalInput")
    d_xT = nc.dram_tensor("xT", [NF, NPC], f32, kind="ExternalInput")
    d_psel = nc.dram_tensor("psel", [P, BPC * G], f32, kind="ExternalInput")
    d_nrme = nc.dram_tensor("nrme", [P, NPC], f32, kind="ExternalInput")
    d_nrmse = nc.dram_tensor("nrmse", [P, NPC], f32, kind="ExternalInput")
    d_W = nc.dram_tensor("W", [P, L * 2 * H], bf16, kind="ExternalInput")
    d_gbT = nc.dram_tensor("gbT", [P, L * 4], f32, kind="ExternalInput")
    d_D = nc.dram_tensor("D", [NF, H], f32, kind="ExternalInput")
    d_baseT = nc.dram_tensor("baseT", [P, 2], f32, kind="ExternalInput")
    d_w1 = nc.dram_tensor("w1", [P, 2 * P], f32, kind="ExternalInput")
    d_w2 = nc.dram_tensor("w2", [P, 64], f32, kind="ExternalInput")
    d_w3 = nc.dram_tensor("w3", [64, 1], f32, kind="ExternalInput")
    d_b1 = nc.dram_tensor("b1", [P, 1], f32, kind="ExternalInput")
    d_b2 = nc.dram_tensor("b2", [64, 1], f32, kind="ExternalInput")
    d_b3 = nc.dram_tensor("b3", [1, 1], f32, kind="ExternalInput")
    d_out = nc.dram_tensor("out", [1, G], f32, kind="ExternalOutput")

    rg = [list(range(NCORE))]

    with tile.TileContext(nc) as tc, ExitStack() as ctx:
        pers = ctx.enter_context(tc.tile_pool(name="pers", bufs=1))
        psH = ctx.enter_context(tc.tile_pool(name="psH", bufs=1, space="PSUM"))
        psB = ctx.enter_context(tc.tile_pool(name="psB", bufs=1, space="PSUM"))
        work = ctx.enter_context(tc.tile_pool(name="work", bufs=2))
        stream = ctx.enter_context(tc.tile_pool(name="stream", bufs=2))
        dram = ctx.enter_context(tc.tile_pool(name="dram", bufs=2, space="DRAM"))

        # ---- persistent SBUF state -------------------------------------
        sel_sb = pers.tile([P, SELW], fp8, tag="sel")
        tab_sb = pers.tile([P, NBLK * H], fp8, tag="tab")
        nrme_sb = pers.tile([P, NPC], f32, tag="nrme")
        nrmse_sb = pers.tile([P, NPC], f32, tag="nrmse")
        W_sb = pers.tile([P, L * 2 * H], bf16, tag="W")
        gbT_sb = pers.tile([P, L * 4], f32, tag="gbT")
        D_sb = pers.tile([NF, H], f32, tag="D")
        baseT_sb = pers.tile([P, 2], f32, tag="baseT")
        w1_sb = pers.tile([P, 2 * P], f32, tag="w1")
        w2_sb = pers.tile([P, 64], f32, tag="w2")
        w3_sb = pers.tile([64, 1], f32, tag="w3")
        b1_sb = pers.tile([P, 1], f32, tag="b1")
        b2_sb = pers.tile([64, 1], f32, tag="b2")
        b3_sb = pers.tile([1, 1], f32, tag="b3")

        hT_sb = pers.tile([P, 2 * NPC], f32, tag="hT")      # halves side by side
        tT_sb = pers.tile([P, 2 * NPC], f32, tag="tT")
        sq_sb = pers.tile([P, NPC], f32, tag="sqs")         # scratch for t^2
        hsT_sb = pers.tile([P, 2 * NPC], bf16, tag="hsT")
        ag_sb = pers.tile([P, BPC * H], fp8, tag="ag")
        stat_sb = pers.tile([P, 4], f32, tag="stat")
        ac_sb = pers.tile([P, 8], f32, tag="ac")            # mu0 mu1 a0 a1 c0 c1 tmp
        ident_f = pers.tile([P, P], f32, tag="ident")

        # ---- DRAM bounce buffers ---------------------------------------
        ag_in = dram.tile([P, BPC * H], fp8, tag="ag_in")
        ag_outs = [dram.tile([NCORE * P, BPC * H], fp8, tag=f"ag_out{l}",
                             addr_space="Shared", name=f"ag_out{l}")
                   for l in range(L)]
        ar_in = dram.tile([P, 4], f32, tag="ar_in")
        ar_outs = [dram.tile([P, 4], f32, tag=f"ar_out{l}",
                             addr_space="Shared", name=f"ar_out{l}")
                   for l in range(L)]
        pr_in = dram.tile([2 * P, G], f32, tag="pr_in")
        pr_out = dram.tile([2 * P, G], f32, tag="pr_out", addr_space="Shared")

        # ---- input loads (small first; 13MB sel last) -------------------
        xT_sb = stream.tile([NF, NPC], f32, tag="xT_sb")
        nc.sync.dma_start(out=xT_sb[:], in_=d_xT[:])
        for t, d in [(nrme_sb, d_nrme), (nrmse_sb, d_nrmse), (W_sb, d_W),
                     (gbT_sb, d_gbT), (D_sb, d_D), (baseT_sb, d_baseT),
                     (w1_sb, d_w1), (w2_sb, d_w2), (w3_sb, d_w3),
                     (b1_sb, d_b1), (b2_sb, d_b2), (b3_sb, d_b3)]:
            nc.sync.dma_start(out=t[:], in_=d[:])
        nc.sync.dma_start(out=sel_sb[:], in_=d_sel[:])
        make_identity(nc, ident_f[:])

        def hT(half):
            return hT_sb[:, half * NPC:(half + 1) * NPC]

        def tT(half):
            return tT_sb[:, half * NPC:(half + 1) * NPC]

        def hsT(half):
            return hsT_sb[:, half * NPC:(half + 1) * NPC]

        # ---- encoder: hT = D^T @ xT + baseT -----------------------------
        psHT = [psH.tile([P, NPC], f32, tag=f"h{i}", name=f"psHT{i}")
                for i in range(2)]
        for half in range(2):
            for (off, ln) in CHUNKS:
                nc.tensor.matmul(out=psHT[half][:, off:off + ln],
                                 lhsT=D_sb[:, half * P:(half + 1) * P],
                                 rhs=xT_sb[:, off:off + ln],
                                 start=True, stop=True)
            nc.vector.tensor_scalar_add(hT(half), psHT[half][:],
                                        baseT_sb[:, half:half + 1])
            # hs = h * nrm (bf16 for the GEMM)
            nc.vector.tensor_tensor(out=hsT(half), in0=hT(half),
                                    in1=nrme_sb[:], op=OP.mult)

        # ---- layers -----------------------------------------------------
        for l in range(L):
            # GEMM hws = hs @ W[l] per dst block, cast to fp8 table shard
            for nb in range(BPC):
                ps_g = psB.tile([P, H], f32, tag="mm" if nb % 2 == 0 else "mm2",
                                name=f"ps_g{nb}")
                for half in range(2):
                    nc.tensor.matmul(
                        out=ps_g[:],
                        lhsT=hsT(half)[:, nb * P:(nb + 1) * P],
                        rhs=W_sb[:, (l * 2 + half) * H:(l * 2 + half + 1) * H],
                        start=(half == 0), stop=(half == 1))
                nc.scalar.activation(out=ag_sb[:, nb * H:(nb + 1) * H],
                                     in_=ps_g[:], func=FT.Copy, scale=FP8S)
            nc.sync.dma_start(out=ag_in[:], in_=ag_sb[:])
            nc.gpsimd.collective_compute(
                "AllGather", OP.bypass, replica_groups=rg,
                ins=[ag_in[:]], outs=[ag_outs[l][:]])
            for r in range(NCORE):
                nc.sync.dma_start(
                    out=tab_sb[:, r * BPC * H:(r + 1) * BPC * H],
                    in_=ag_outs[l][r * P:(r + 1) * P, :])

            # aggregation: psHT[half] += tab_pair^T (DR) @ sel chunks.
            # half 0 finishes all pairs first so its stats (DVE) overlap
            # half 1's matmuls.
            psHT = [psH.tile([P, NPC], f32, tag=f"h{i}", name=f"psT{l}{i}")
                    for i in range(2)]
            for half in range(2):
                for k in range(NPAIR):
                    tpair = tab_sb[:, 2 * k * H:(2 * k + 2) * H].rearrange(
                        "p (two h) -> p two h", two=2)
                    spair = sel_sb[:, k * 2 * NPC:(k + 1) * 2 * NPC].rearrange(
                        "p (two d) -> p two d", two=2)
                    lhsT = tpair[:, :, half * P:(half + 1) * P]
                    for (off, ln) in CHUNKS:
                        nc.tensor.matmul(
                            out=psHT[half][:, off:off + ln],
                            lhsT=lhsT,
                            rhs=spair[:, :, off:off + ln],
                            start=(k == 0), stop=(k == NPAIR - 1),
                            perf_mode=DRM)
                # t = ps*nrm/S (+col-sum), sq = t*t (+col-sumsq) — fused DVE
                nc.vector.scalar_tensor_tensor(
                    out=tT(half), in0=psHT[half][:], scalar=1.0,
                    in1=nrmse_sb[:], op0=OP.mult, op1=OP.mult,
                    accum_out=stat_sb[:, half:half + 1])
                nc.vector.scalar_tensor_tensor(
                    out=sq_sb[:], in0=tT(half), scalar=1.0,
                    in1=tT(half), op0=OP.mult, op1=OP.mult,
                    accum_out=stat_sb[:, 2 + half:3 + half])
            nc.sync.dma_start(out=ar_in[:], in_=stat_sb[:])
            nc.gpsimd.collective_compute(
                "AllReduce", OP.add, replica_groups=rg,
                ins=[ar_in[:]], outs=[ar_outs[l][:]])
            nc.sync.dma_start(out=stat_sb[:], in_=ar_outs[l][:])

            # per-partition BN coeffs: a = gamma*istd, c = beta - mu*a
            mu2 = ac_sb[:, 0:2]
            var2 = ac_sb[:, 2:4]
            a2 = ac_sb[:, 4:6]
            c2 = ac_sb[:, 6:8]
            nc.vector.tensor_scalar_mul(mu2, stat_sb[:, 0:2], 1.0 / N)
            nc.vector.tensor_scalar_mul(var2, stat_sb[:, 2:4], 1.0 / N)
            nc.vector.tensor_tensor(out=a2, in0=mu2, in1=mu2, op=OP.mult)
            nc.vector.tensor_tensor(out=var2, in0=var2, in1=a2, op=OP.subtract)
            nc.vector.tensor_scalar_add(var2, var2, BN_EPS)
            nc.vector.reciprocal(out=var2, in_=var2)
            nc.scalar.activation(out=var2, in_=var2, func=FT.Sqrt)  # istd
            nc.vector.tensor_tensor(out=a2, in0=var2,
                                    in1=gbT_sb[:, l * 4:l * 4 + 2], op=OP.mult)
            nc.vector.tensor_tensor(out=c2, in0=mu2, in1=a2, op=OP.mult)
            nc.vector.tensor_tensor(out=c2, in0=gbT_sb[:, l * 4 + 2:l * 4 + 4],
                                    in1=c2, op=OP.subtract)

            # h += relu(a*t + c) ; hs = h*nrm for next GEMM.  Split into two
            # node-chunks so the next layer's GEMM (which consumes hs slices
            # per 128-node block) can start while the second chunk applies.
            for (off, ln) in [(0, 5 * P), (5 * P, 5 * P)]:
                for half in range(2):
                    r_t = work.tile([P, ln], f32, tag="r_t")
                    nc.scalar.activation(out=r_t[:],
                                         in_=tT(half)[:, off:off + ln],
                                         func=FT.Relu,
                                         scale=ac_sb[:, 4 + half:5 + half],
                                         bias=ac_sb[:, 6 + half:7 + half])
                    nc.vector.tensor_tensor(out=hT(half)[:, off:off + ln],
                                            in0=hT(half)[:, off:off + ln],
                                            in1=r_t[:], op=OP.add)
                    if l < L - 1:
                        nc.vector.tensor_tensor(
                            out=hsT(half)[:, off:off + ln],
                            in0=hT(half)[:, off:off + ln],
                            in1=nrme_sb[:, off:off + ln], op=OP.mult)

        # ---- pooling: transpose hT blocks, one-hot matmul ---------------
        ps_p0 = psB.tile([P, G], f32, tag="mm")
        ps_p1 = psB.tile([P, G], f32, tag="mm2")
        hblk = [work.tile([P, P], f32, tag=f"hp{i}", name=f"hblk{i}")
                for i in range(2)]
        for nb in range(BPC):
            psel_t = stream.tile([P, G], f32, tag="psel_t")
            nc.sync.dma_start(out=psel_t[:], in_=d_psel[:, nb * G:(nb + 1) * G])
            for half in range(2):
                ps_tr = psH.tile([P, P], f32, tag=f"h{half}", name=f"ptr{half}")
                nc.tensor.transpose(out=ps_tr[:],
                                    in_=hT(half)[:, nb * P:(nb + 1) * P],
                                    identity=ident_f[:])
                nc.vector.tensor_copy(out=hblk[half][:], in_=ps_tr[:])
            nc.tensor.matmul(out=ps_p0[:], lhsT=hblk[0][:], rhs=psel_t[:],
                             start=(nb == 0), stop=(nb == BPC - 1))
            nc.tensor.matmul(out=ps_p1[:], lhsT=hblk[1][:], rhs=psel_t[:],
                             start=(nb == 0), stop=(nb == BPC - 1))
        g0 = work.tile([P, G], f32, tag="g0")
        g1 = work.tile([P, G], f32, tag="g1")
        nc.vector.tensor_copy(out=g0[:], in_=ps_p0[:])
        nc.vector.tensor_copy(out=g1[:], in_=ps_p1[:])
        nc.sync.dma_start(out=pr_in[0:P, :], in_=g0[:])
        nc.sync.dma_start(out=pr_in[P:2 * P, :], in_=g1[:])
        nc.gpsimd.collective_compute(
            "AllReduce", OP.add, replica_groups=rg,
            ins=[pr_in[:]], outs=[pr_out[:]])
        nc.sync.dma_start(out=g0[:], in_=pr_out[0:P, :])
        nc.sync.dma_start(out=g1[:], in_=pr_out[P:2 * P, :])

        # MLP head (weights as lhsT, graphs along free dim)
        ps1 = psB.tile([P, G], f32, tag="mm")
        nc.tensor.matmul(out=ps1[:], lhsT=w1_sb[:, 0:P], rhs=g0[:],
                         start=True, stop=False)
        nc.tensor.matmul(out=ps1[:], lhsT=w1_sb[:, P:2 * P], rhs=g1[:],
                         start=False, stop=True)
        y1 = work.tile([P, G], f32, tag="y1")
        nc.scalar.activation(out=y1[:], in_=ps1[:], func=FT.Relu,
                             bias=b1_sb[:, 0:1])
        ps2 = psB.tile([64, G], f32, tag="mm2")
        nc.tensor.matmul(out=ps2[:], lhsT=w2_sb[:], rhs=y1[:],
                         start=True, stop=True)
        y2 = work.tile([64, G], f32, tag="y2")
        nc.scalar.activation(out=y2[:], in_=ps2[:], func=FT.Relu,
                             bias=b2_sb[:, 0:1])
        ps3 = psB.tile([1, G], f32, tag="mm")
        nc.tensor.matmul(out=ps3[:], lhsT=w3_sb[:], rhs=y2[:],
                         start=True, stop=True)
        y3 = work.tile([1, G], f32, tag="y3")
        nc.vector.tensor_scalar_add(y3[:], ps3[:], b3_sb[0:1, 0:1])
        nc.sync.dma_start(out=d_out[:], in_=y3[:])

    nc.compile()
    return nc


# --------------------------------------------------------------------------
# entry point
# --------------------------------------------------------------------------

def kernel(x, edge_index, batch_ids, emb, W, b, gamma, beta,
           mlp_W1, mlp_b1, mlp_W2, mlp_b2, mlp_W3, mlp_b3,
           _trace=False, _trace_kwargs=None):
    in_maps = _preprocess(x, edge_index, batch_ids, emb, W, gamma, beta,
                          mlp_W1, mlp_b1, mlp_W2, mlp_b2, mlp_W3, mlp_b3)
    if "nc" not in _compiled:
        _compiled["nc"] = _build()
    nc = _compiled["nc"]
    kw = {}
    if _trace:
        kw = dict(trace=True, **(_trace_kwargs or {}))
    res = run_bass_kernel_spmd(nc, in_maps, core_ids=list(range(NCORE)), **kw)
    out = np.asarray(res.results[0]["out"], np.float32).reshape(G, 1)
    kernel._last_results = res
    return out


# revision 27
# speedup vs baseline: 4.3654x; 1.0135x over previous
"""Trainium2 Bass kernel for HIVNet GCN message passing (8-core SPMD).

Strategy (v3 — transposed dense aggregation, table-stationary):
  - Pad N=10000 nodes to 10240 = 80 blocks x 128; core c owns 10 dst-blocks
    (1280 nodes).  Node state h is kept TRANSPOSED: hT[half][h, n] with the
    hidden dim on partitions (2 halves of 128) and the core's 1280 nodes on
    the free axis.
  - Per layer: GEMM hws = hs @ W[l] produces node-major [128, 256] blocks
    (lhsT = hsT directly, no transposes), cast to fp8e4m3 (x32), AllGather'd
    into a DRAM table; remote shards are loaded into SBUF.
  - Aggregation (TensorE, DoubleRow fp8): stationary = table block-pair
    [128, 2, 128-H-half], moving = host-built dense edge-count matrix
    sel[p, i, dst] over the core's 1280 dst in 512-wide chunks.  psHT[half]
    [128, 1280] accumulates over all 40 pairs; the core's OWN 5 pairs read
    the local fp8 copy and overlap the AllGather of the rest.
  - BN: fused DVE tensor_tensor_reduce produces t = ps*nrm/S (+sum) and
    t^2 (+sumsq) in two passes; [128, 4] AllReduce; a,c are per-partition so
    apply is a single fused ACT Relu(a*t + c) per half + residual add.
  - Readout: transpose h once, graph mean-pool one-hots (1/cnt folded),
    2*128-row AllReduce, 3-layer MLP.
"""

import sys

sys.path.insert(0, "/opt/trn_rl_repo")

from contextlib import ExitStack

import numpy as np
import ml_dtypes

from concourse import bass, mybir, bacc, tile
from concourse.bass_utils import run_bass_kernel_spmd
from concourse.masks import make_identity

NCORE = 8
P = 128
H = 256
L = 4
NF = 9
G = 256
N = 10000
BPC = 10                # dst blocks per core
NPC = BPC * P           # 1280 nodes per core
NPAD = NCORE * NPC      # 10240
NBLK = NPAD // P        # 80 src blocks
NPAIR = NBLK // 2       # 40 src block pairs (DoubleRow K=256)
BN_EPS = 1e-5
FP8S = 32.0             # fp8 table scale
CHUNKS = [(0, 512), (512, 512), (1024, 256)]   # dst chunks (<=512 f32 PSUM bank)

f32 = mybir.dt.float32
bf16 = mybir.dt.bfloat16
fp8 = mybir.dt.float8e4
fp8np = mybir.dt.np(mybir.dt.float8e4)
bfnp = ml_dtypes.bfloat16

FT = mybir.ActivationFunctionType
OP = mybir.AluOpType
DRM = mybir.MatmulPerfMode.DoubleRow

_compiled = {}


# --------------------------------------------------------------------------
# host-side structural preprocessing
# --------------------------------------------------------------------------

def _preprocess(x, edge_index, batch_ids, emb, W, gamma, beta,
                mlp_W1, mlp_b1, mlp_W2, mlp_b2, mlp_W3, mlp_b3):
    src = np.asarray(edge_index[0], np.int64)
    dst = np.asarray(edge_index[1], np.int64)
    src_all = np.concatenate([src, np.arange(N, dtype=np.int64)])
    dst_all = np.concatenate([dst, np.arange(N, dtype=np.int64)])

    deg = np.bincount(dst_all, minlength=NPAD).astype(np.float64)
    nrm_full = np.zeros(NPAD, np.float32)
    nrm_full[:NPAD] = 1.0 / np.sqrt(np.maximum(deg, 1.0))
    nrm_full[deg == 0] = 0.0

    # dense per-core sel (moving operand): [128 p, 40 pair, 2 i, 1280 dst]
    p_idx = (src_all % P).astype(np.int64)
    blk = src_all // P
    k_idx = blk // 2
    i_idx = blk % 2
    core = dst_all // NPC
    d_idx = dst_all % NPC
    sels = []
    for c in range(NCORE):
        m = core == c
        selc = np.zeros((P, NPAIR, 2, NPC), np.float32)
        np.add.at(selc, (p_idx[m], k_idx[m], i_idx[m], d_idx[m]), 1.0)
        sels.append(selc.reshape(P, NPAIR * 2 * NPC).astype(fp8np))

    # graph pool one-hot with 1/count folded in
    bids = np.asarray(batch_ids, np.int64)
    cnt = np.bincount(bids, minlength=G).astype(np.float32)
    inv = 1.0 / np.maximum(cnt, 1.0)
    psel_full = np.zeros((NPAD, G), np.float32)
    psel_full[np.arange(N), bids] = inv[bids]

    x_np = np.zeros((NPAD, NF), np.float32)
    x_np[:N] = np.asarray(x, np.float64)

    # encoder prep on host: D = emb1 - emb0, base = sum_f emb0[f]
    embf = np.asarray(emb, np.float32)
    D = np.ascontiguousarray(embf[:, 1, :] - embf[:, 0, :])       # [9, 256]
    baseT = np.ascontiguousarray(embf[:, 0, :].sum(0).reshape(2, P).T)  # [128,2]

    Wf = np.asarray(W, np.float32)
    W_lhsT = Wf.reshape(L, 2, P, H).transpose(2, 0, 1, 3).reshape(P, L * 2 * H)
    # gamma/beta transposed per half: [128, L*4] = (g0,g1,b0,b1) per layer
    gaT = np.asarray(gamma, np.float32).reshape(L, 2, P)
    beT = np.asarray(beta, np.float32).reshape(L, 2, P)
    gbT = np.concatenate([gaT, beT], axis=1).transpose(2, 0, 1).reshape(P, L * 4)

    w1 = np.asarray(mlp_W1, np.float32).reshape(2, P, P).transpose(1, 0, 2).reshape(P, 2 * P)
    w2 = np.asarray(mlp_W2, np.float32)
    w3 = np.asarray(mlp_W3, np.float32)
    b1 = np.asarray(mlp_b1, np.float32).reshape(P, 1)
    b2 = np.asarray(mlp_b2, np.float32).reshape(64, 1)
    b3 = np.asarray(mlp_b3, np.float32).reshape(1, 1)

    in_maps = []
    for c in range(NCORE):
        lo, hi = c * NPC, (c + 1) * NPC
        nrmc = nrm_full[lo:hi]
        nrm_exp = np.broadcast_to(nrmc, (P, NPC)).copy()          # [128,1280]
        nrms_exp = (nrm_exp / FP8S).astype(np.float32)

        pselc = psel_full[lo:hi].reshape(BPC, P, G)
        pselc = np.ascontiguousarray(pselc.transpose(1, 0, 2)).reshape(P, BPC * G)

        xTc = np.ascontiguousarray(x_np[lo:hi].T)                 # [9, 1280]

        in_maps.append(dict(
            selw=sels[c], xT=xTc, psel=pselc,
            nrme=nrm_exp, nrmse=nrms_exp,
            W=W_lhsT.astype(bfnp), gbT=gbT, D=D, baseT=baseT,
            w1=w1, w2=w2, w3=w3, b1=b1, b2=b2, b3=b3,
        ))
    return in_maps


# --------------------------------------------------------------------------
# device program
# --------------------------------------------------------------------------

def _build():
    SELW = NPAIR * 2 * NPC          # 102400
    nc = bacc.Bacc(None, target_bir_lowering=False)

    d_sel = nc.dram_tensor("selw", [P, SELW], fp8, kind="ExternalInput")
    d_xT = nc.dram_tensor("xT", [NF, NPC], f32, kind="ExternalInput")
    d_psel = nc.dram_tensor("psel", [P, BPC * G], f32, kind="ExternalInput")
    d_nrme = nc.dram_tensor("nrme", [P, NPC], f32, kind="ExternalInput")
    d_nrmse = nc.dram_tensor("nrmse", [P, NPC], f32, kind="ExternalInput")
    d_W = nc.dram_tensor("W", [P, L * 2 * H], bf16, kind="ExternalInput")
    d_gbT = nc.dram_tensor("gbT", [P, L * 4], f32, kind="ExternalInput")
    d_D = nc.dram_tensor("D", [NF, H], f32, kind="ExternalInput")
    d_baseT = nc.dram_tensor("baseT", [P, 2], f32, kind="ExternalInput")
    d_w1 = nc.dram_tensor("w1", [P, 2 * P], f32, kind="ExternalInput")
    d_w2 = nc.dram_tensor("w2", [P, 64], f32, kind="ExternalInput")
    d_w3 = nc.dram_tensor("w3", [64, 1], f32, kind="ExternalInput")
    d_b1 = nc.dram_tensor("b1", [P, 1], f32, kind="ExternalInput")
    d_b2 = nc.dram_tensor("b2", [64, 1], f32, kind="ExternalInput")
    d_b3 = nc.dram_tensor("b3", [1, 1], f32, kind="ExternalInput")
    d_out = nc.dram_tensor("out", [1, G], f32, kind="ExternalOutput")

    rg = [list(range(NCORE))]

    with tile.TileContext(nc) as tc, ExitStack() as ctx:
        pers = ctx.enter_context(tc.tile_pool(name="pers", bufs=1))
        psH = ctx.enter_context(tc.tile_pool(name="psH", bufs=1, space="PSUM"))
        psB = ctx.enter_context(tc.tile_pool(name="psB", bufs=1, space="PSUM"))
        work = ctx.enter_context(tc.tile_pool(name="work", bufs=2))
        stream = ctx.enter_context(tc.tile_pool(name="stream", bufs=2))
        dram = ctx.enter_context(tc.tile_pool(name="dram", bufs=2, space="DRAM"))

        # ---- persistent SBUF state -------------------------------------
        sel_sb = pers.tile([P, SELW], fp8, tag="sel")
        tab_sb = pers.tile([P, NBLK * H], fp8, tag="tab")
        nrme_sb = pers.tile([P, NPC], f32, tag="nrme")
        nrmse_sb = pers.tile([P, NPC], f32, tag="nrmse")
        W_sb = pers.tile([P, L * 2 * H], bf16, tag="W")
        gbT_sb = pers.tile([P, L * 4], f32, tag="gbT")
        D_sb = pers.tile([NF, H], f32, tag="D")
        baseT_sb = pers.tile([P, 2], f32, tag="baseT")
        w1_sb = pers.tile([P, 2 * P], f32, tag="w1")
        w2_sb = pers.tile([P, 64], f32, tag="w2")
        w3_sb = pers.tile([64, 1], f32, tag="w3")
        b1_sb = pers.tile([P, 1], f32, tag="b1")
        b2_sb = pers.tile([64, 1], f32, tag="b2")
        b3_sb = pers.tile([1, 1], f32, tag="b3")

        hT_sb = pers.tile([P, 2 * NPC], f32, tag="hT")      # halves side by side
        tT_sb = pers.tile([P, 2 * NPC], f32, tag="tT")
        sq_sb = pers.tile([P, NPC], f32, tag="sqs")         # scratch for t^2
        hsT_sb = pers.tile([P, 2 * NPC], bf16, tag="hsT")
        ag_sb = pers.tile([P, BPC * H], fp8, tag="ag")
        stat_sb = pers.tile([P, 4], f32, tag="stat")
        ac_sb = pers.tile([P, 8], f32, tag="ac")            # mu0 mu1 a0 a1 c0 c1 tmp
        ident_f = pers.tile([P, P], f32, tag="ident")

        # ---- DRAM bounce buffers ---------------------------------------
        B1, B2 = 4, 6          # split AllGather: first 4 blocks / last 6
        ag_in1 = dram.tile([P, B1 * H], fp8, tag="ag_in1")
        ag_in2 = dram.tile([P, B2 * H], fp8, tag="ag_in2")
        ag1_outs = [dram.tile([NCORE * P, B1 * H], fp8, tag=f"ag1_out{l}",
                              addr_space="Shared", name=f"ag1_out{l}")
                    for l in range(L)]
        ag2_outs = [dram.tile([NCORE * P, B2 * H], fp8, tag=f"ag2_out{l}",
                              addr_space="Shared", name=f"ag2_out{l}")
                    for l in range(L)]
        ar_in = dram.tile([P, 4], f32, tag="ar_in")
        ar_outs = [dram.tile([P, 4], f32, tag=f"ar_out{l}",
                             addr_space="Shared", name=f"ar_out{l}")
                   for l in range(L)]
        pr_in = dram.tile([2 * P, G], f32, tag="pr_in")
        pr_out = dram.tile([2 * P, G], f32, tag="pr_out", addr_space="Shared")

        # ---- input loads (small first; 13MB sel last) -------------------
        xT_sb = stream.tile([NF, NPC], f32, tag="xT_sb")
        nc.sync.dma_start(out=xT_sb[:], in_=d_xT[:])
        for t, d in [(nrme_sb, d_nrme), (nrmse_sb, d_nrmse), (W_sb, d_W),
                     (gbT_sb, d_gbT), (D_sb, d_D), (baseT_sb, d_baseT),
                     (w1_sb, d_w1), (w2_sb, d_w2), (w3_sb, d_w3),
                     (b1_sb, d_b1), (b2_sb, d_b2), (b3_sb, d_b3)]:
            nc.sync.dma_start(out=t[:], in_=d[:])
        # 13MB sel load rides the scalar HWDGE ring so it doesn't block the
        # sync ring (ag_in upload, tab section loads) during layer 0.
        nc.scalar.dma_start(out=sel_sb[:], in_=d_sel[:])
        make_identity(nc, ident_f[:])

        def hT(half):
            return hT_sb[:, half * NPC:(half + 1) * NPC]

        def tT(half):
            return tT_sb[:, half * NPC:(half + 1) * NPC]

        def hsT(half):
            return hsT_sb[:, half * NPC:(half + 1) * NPC]

        # ---- encoder: hT = D^T @ xT + baseT -----------------------------
        psHT = [psH.tile([P, NPC], f32, tag=f"h{i}", name=f"psHT{i}")
                for i in range(2)]
        for half in range(2):
            for (off, ln) in CHUNKS:
                nc.tensor.matmul(out=psHT[half][:, off:off + ln],
                                 lhsT=D_sb[:, half * P:(half + 1) * P],
                                 rhs=xT_sb[:, off:off + ln],
                                 start=True, stop=True)
            nc.vector.tensor_scalar_add(hT(half), psHT[half][:],
                                        baseT_sb[:, half:half + 1])
            # hs = h * nrm (bf16 for the GEMM)
            nc.vector.tensor_tensor(out=hsT(half), in0=hT(half),
                                    in1=nrme_sb[:], op=OP.mult)

        # ---- layers -----------------------------------------------------
        for l in range(L):
            # GEMM hws = hs @ W[l] per dst block, cast to fp8 table shard
            for nb in range(BPC):
                ps_g = psB.tile([P, H], f32, tag="mm" if nb % 2 == 0 else "mm2",
                                name=f"ps_g{nb}")
                for half in range(2):
                    nc.tensor.matmul(
                        out=ps_g[:],
                        lhsT=hsT(half)[:, nb * P:(nb + 1) * P],
                        rhs=W_sb[:, (l * 2 + half) * H:(l * 2 + half + 1) * H],
                        start=(half == 0), stop=(half == 1))
                nc.scalar.activation(out=ag_sb[:, nb * H:(nb + 1) * H],
                                     in_=ps_g[:], func=FT.Copy, scale=FP8S)
            nc.sync.dma_start(out=ag_in1[:], in_=ag_sb[:, 0:B1 * H])
            nc.gpsimd.collective_compute(
                "AllGather", OP.bypass, replica_groups=rg,
                ins=[ag_in1[:]], outs=[ag1_outs[l][:]])
            nc.sync.dma_start(out=ag_in2[:], in_=ag_sb[:, B1 * H:])
            nc.gpsimd.collective_compute(
                "AllGather", OP.bypass, replica_groups=rg,
                ins=[ag_in2[:]], outs=[ag2_outs[l][:]])
            for r in range(NCORE):
                nc.sync.dma_start(
                    out=tab_sb[:, (r * BPC) * H:(r * BPC + B1) * H],
                    in_=ag1_outs[l][r * P:(r + 1) * P, :])
            for r in range(NCORE):
                nc.sync.dma_start(
                    out=tab_sb[:, (r * BPC + B1) * H:(r + 1) * BPC * H],
                    in_=ag2_outs[l][r * P:(r + 1) * P, :])

            # aggregation: psHT[half] += tab_pair^T (DR) @ sel chunks.
            # half 0 finishes all pairs first so its stats (DVE) overlap
            # half 1's matmuls.
            psHT = [psH.tile([P, NPC], f32, tag=f"h{i}", name=f"psT{l}{i}")
                    for i in range(2)]
            # pairs covered by AG1 (first 2 pairs of each rank) first, so
            # aggregation starts while AG2 is still in flight
            korder = [k for k in range(NPAIR) if k % 5 < B1 // 2] + \
                     [k for k in range(NPAIR) if k % 5 >= B1 // 2]
            for half in range(2):
                for ki, k in enumerate(korder):
                    tpair = tab_sb[:, 2 * k * H:(2 * k + 2) * H].rearrange(
                        "p (two h) -> p two h", two=2)
                    spair = sel_sb[:, k * 2 * NPC:(k + 1) * 2 * NPC].rearrange(
                        "p (two d) -> p two d", two=2)
                    lhsT = tpair[:, :, half * P:(half + 1) * P]
                    for (off, ln) in CHUNKS:
                        nc.tensor.matmul(
                            out=psHT[half][:, off:off + ln],
                            lhsT=lhsT,
                            rhs=spair[:, :, off:off + ln],
                            start=(ki == 0), stop=(ki == NPAIR - 1),
                            perf_mode=DRM)
                # t = ps*nrm/S (+col-sum), sq = t*t (+col-sumsq) — fused DVE
                nc.vector.scalar_tensor_tensor(
                    out=tT(half), in0=psHT[half][:], scalar=1.0,
                    in1=nrmse_sb[:], op0=OP.mult, op1=OP.mult,
                    accum_out=stat_sb[:, half:half + 1])
                nc.vector.scalar_tensor_tensor(
                    out=sq_sb[:], in0=tT(half), scalar=1.0,
                    in1=tT(half), op0=OP.mult, op1=OP.mult,
                    accum_out=stat_sb[:, 2 + half:3 + half])
            nc.sync.dma_start(out=ar_in[:], in_=stat_sb[:])
            nc.gpsimd.collective_compute(
                "AllReduce", OP.add, replica_groups=rg,
                ins=[ar_in[:]], outs=[ar_outs[l][:]])
            nc.sync.dma_start(out=stat_sb[:], in_=ar_outs[l][:])

            # per-partition BN coeffs: a = gamma*istd, c = beta - mu*a
            mu2 = ac_sb[:, 0:2]
            var2 = ac_sb[:, 2:4]
            a2 = ac_sb[:, 4:6]
            c2 = ac_sb[:, 6:8]
            nc.vector.tensor_scalar_mul(mu2, stat_sb[:, 0:2], 1.0 / N)
            nc.vector.tensor_scalar_mul(var2, stat_sb[:, 2:4], 1.0 / N)
            nc.vector.tensor_tensor(out=a2, in0=mu2, in1=mu2, op=OP.mult)
            nc.vector.tensor_tensor(out=var2, in0=var2, in1=a2, op=OP.subtract)
            nc.vector.tensor_scalar_add(var2, var2, BN_EPS)
            nc.vector.reciprocal(out=var2, in_=var2)
            nc.scalar.activation(out=var2, in_=var2, func=FT.Sqrt)  # istd
            nc.vector.tensor_tensor(out=a2, in0=var2,
                                    in1=gbT_sb[:, l * 4:l * 4 + 2], op=OP.mult)
            nc.vector.tensor_tensor(out=c2, in0=mu2, in1=a2, op=OP.mult)
            nc.vector.tensor_tensor(out=c2, in0=gbT_sb[:, l * 4 + 2:l * 4 + 4],
                                    in1=c2, op=OP.subtract)

            # h += relu(a*t + c) ; hs = h*nrm for next GEMM.  Split into two
            # node-chunks so the next layer's GEMM (which consumes hs slices
            # per 128-node block) can start while the second chunk applies.
            for (off, ln) in [(0, 5 * P), (5 * P, 5 * P)]:
                for half in range(2):
                    r_t = work.tile([P, ln], f32, tag="r_t")
                    nc.scalar.activation(out=r_t[:],
                                         in_=tT(half)[:, off:off + ln],
                                         func=FT.Relu,
                                         scale=ac_sb[:, 4 + half:5 + half],
                                         bias=ac_sb[:, 6 + half:7 + half])
                    nc.vector.tensor_tensor(out=hT(half)[:, off:off + ln],
                                            in0=hT(half)[:, off:off + ln],
                                            in1=r_t[:], op=OP.add)
                    if l < L - 1:
                        nc.vector.tensor_tensor(
                            out=hsT(half)[:, off:off + ln],
                            in0=hT(half)[:, off:off + ln],
                            in1=nrme_sb[:, off:off + ln], op=OP.mult)

        # ---- pooling: transpose hT blocks, one-hot matmul ---------------
        ps_p0 = psB.tile([P, G], f32, tag="mm")
        ps_p1 = psB.tile([P, G], f32, tag="mm2")
        hblk = [work.tile([P, P], f32, tag=f"hp{i}", name=f"hblk{i}")
                for i in range(2)]
        for nb in range(BPC):
            psel_t = stream.tile([P, G], f32, tag="psel_t")
            nc.sync.dma_start(out=psel_t[:], in_=d_psel[:, nb * G:(nb + 1) * G])
            for half in range(2):
                ps_tr = psH.tile([P, P], f32, tag=f"h{half}", name=f"ptr{half}")
                nc.tensor.transpose(out=ps_tr[:],
                                    in_=hT(half)[:, nb * P:(nb + 1) * P],
                                    identity=ident_f[:])
                nc.vector.tensor_copy(out=hblk[half][:], in_=ps_tr[:])
            nc.tensor.matmul(out=ps_p0[:], lhsT=hblk[0][:], rhs=psel_t[:],
                             start=(nb == 0), stop=(nb == BPC - 1))
            nc.tensor.matmul(out=ps_p1[:], lhsT=hblk[1][:], rhs=psel_t[:],
                             start=(nb == 0), stop=(nb == BPC - 1))
        g0 = work.tile([P, G], f32, tag="g0")
        g1 = work.tile([P, G], f32, tag="g1")
        nc.vector.tensor_copy(out=g0[:], in_=ps_p0[:])
        nc.vector.tensor_copy(out=g1[:], in_=ps_p1[:])
        nc.sync.dma_start(out=pr_in[0:P, :], in_=g0[:])
        nc.sync.dma_start(out=pr_in[P:2 * P, :], in_=g1[:])
        nc.gpsimd.collective_compute(
            "AllReduce", OP.add, replica_groups=rg,
            ins=[pr_in[:]], outs=[pr_out[:]])
        nc.sync.dma_start(out=g0[:], in_=pr_out[0:P, :])
        nc.sync.dma_start(out=g1[:], in_=pr_out[P:2 * P, :])

        # MLP head (weights as lhsT, graphs along free dim)
        ps1 = psB.tile([P, G], f32, tag="mm")
        nc.tensor.matmul(out=ps1[:], lhsT=w1_sb[:, 0:P], rhs=g0[:],
                         start=True, stop=False)
        nc.tensor.matmul(out=ps1[:], lhsT=w1_sb[:, P:2 * P], rhs=g1[:],
                         start=False, stop=True)
        y1 = work.tile([P, G], f32, tag="y1")
        nc.scalar.activation(out=y1[:], in_=ps1[:], func=FT.Relu,
                             bias=b1_sb[:, 0:1])
        ps2 = psB.tile([64, G], f32, tag="mm2")
        nc.tensor.matmul(out=ps2[:], lhsT=w2_sb[:], rhs=y1[:],
                         start=True, stop=True)
        y2 = work.tile([64, G], f32, tag="y2")
        nc.scalar.activation(out=y2[:], in_=ps2[:], func=FT.Relu,
                             bias=b2_sb[:, 0:1])
        ps3 = psB.tile([1, G], f32, tag="mm")
        nc.tensor.matmul(out=ps3[:], lhsT=w3_sb[:], rhs=y2[:],
                         start=True, stop=True)
        y3 = work.tile([1, G], f32, tag="y3")
        nc.vector.tensor_scalar_add(y3[:], ps3[:], b3_sb[0:1, 0:1])
        nc.sync.dma_start(out=d_out[:], in_=y3[:])

    nc.compile()
    return nc


# --------------------------------------------------------------------------
# entry point
# --------------------------------------------------------------------------

def kernel(x, edge_index, batch_ids, emb, W, b, gamma, beta,
           mlp_W1, mlp_b1, mlp_W2, mlp_b2, mlp_W3, mlp_b3,
           _trace=False, _trace_kwargs=None):
    in_maps = _preprocess(x, edge_index, batch_ids, emb, W, gamma, beta,
                          mlp_W1, mlp_b1, mlp_W2, mlp_b2, mlp_W3, mlp_b3)
    if "nc" not in _compiled:
        _compiled["nc"] = _build()
    nc = _compiled["nc"]
    kw = {}
    if _trace:
        kw = dict(trace=True, **(_trace_kwargs or {}))
    res = run_bass_kernel_spmd(nc, in_maps, core_ids=list(range(NCORE)), **kw)
    out = np.asarray(res.results[0]["out"], np.float32).reshape(G, 1)
    kernel._last_results = res
    return out
